# revision 59
# baseline (speedup 1.0000x reference)
"""Trainium2 Bass kernel for a dense transformer block (LN-attn-LN-MLP).

Sharding: core pair (2b, 2b+1) handles batch b. Each core computes 8 of the
16 attention heads over the full 2048-token sequence (head/tensor parallel),
then the pair ReduceScatters the partial c_proj output over tokens; the MLP
runs on each core's 1024-token half. All activations on chip are kept
feature-major [features, tokens] so no transposes are needed; the host
pre-transposes x and post-transposes the outputs.

Pipeline structure (v4):
 - Phase 1 prefetches the first x chunk ahead of the weights, computes LN1
   stats, q, k and v for all local heads, and stages the residual (bf16,
   with b_proj folded in) while the PE has slack.
 - Phase 2 runs attention query-chunk-major in order (1,3,0,2) so each pair
   ReduceScatter is issued as early as its two chunks allow; per chunk both
   heads of a pair share one PSUM score tile and a single batched exp
   ACTIVATE; ~1/3 of the off-diagonal exp blocks run on the DVE instead via
   a Schraudolph bit-trick exp (int16 bits -> bf16 bitcast) to unload the
   ACT engine that otherwise paces attention. The softmax 1/rowsum chains
   for all four head pairs are software-pipelined: reciprocal chains issue
   inside the head loop, the PE-side broadcast matmuls and the final muls
   run batched at chunk end so the in-order PE queue never waits on them.
   proj partials are written to the collective buffers on the gpsimd queue
   (keeps the Sync DMA queue short for the tiny softmax DMAs).
 - Phase 3 processes token-half 1 then 0 (matching collective completion
   order), adds the residual, runs LN2 (mean subtraction on DVE; gamma in
   the weights, beta/bias on the GELU activation bias) and the MLP per
   512-token chunk; fc1 contracts over exactly 8 k-tiles.
"""

import os
import sys

sys.path.insert(0, "/opt/trn_rl_repo")

import ml_dtypes
import numpy as np

import concourse.bass as bass
import concourse.tile as tile
from concourse import bacc, mybir
from concourse.bass_utils import run_bass_kernel_spmd

F32 = mybir.dt.float32
F32R = mybir.dt.float32r
BF16 = mybir.dt.bfloat16
AF = mybir.ActivationFunctionType
ALU = mybir.AluOpType

# Model dims
C = 1024            # embed
T = 2048            # sequence length (full context per core)
B = 4               # batch
NHEAD = 16
D = 64              # head dim
HL = 8              # local heads per core
TO = 1024           # output tokens per core (after pair ReduceScatter)
CH = 512            # token chunk (matmul free dim)
NCH = T // CH       # 4
KB = 128            # key block
FF = 4 * C          # 4096
EPS = 1e-5

KT_C = C // 128     # 8 k-tiles over embed dim
KS = KT_C + 1       # k-tiles incl. the mu'/ones/zero-pad tail tile (qkv only)
CP = KS * 128       # padded stack rows (1152)
KT_P = HL * D // 128       # proj k-tiles (4; bias folded into residual)
KT_F = FF // 128           # 32
KT_F2 = FF // 128          # fc2 k-tiles (32; bias folded into output add)
FP2 = KT_F2 * 128          # fc2 rows (4096)
FG = 8                     # fc2 weight k-tiles per DMA group


def _r(ap):
    return ap.bitcast(F32R) if ap.dtype == F32 else ap


def _emit(tc, io):
    nc = tc.nc
    xT, xres, wq, wk, wv, wp, wfc, wfc2, out_ap = (
        io["xT"], io["xres"], io["wq"], io["wk"], io["wv"], io["wp"],
        io["wfc"], io["wfc2"], io["out"])

    const = tc.alloc_tile_pool(name="const", bufs=1)
    ones_stage = const.tile([2, T], F32)
    nc.vector.memset(ones_stage[:], 1.0)
    ones_lhs_s = const.tile([128, 1], F32)
    nc.vector.memset(ones_lhs_s[:], 1.0)
    ones_lhs = const.tile([128, 1], F32R)
    nc.vector.tensor_copy(ones_lhs[:], ones_lhs_s[:])
    ones_lhs_bf = const.tile([128, 1], BF16)
    nc.vector.tensor_copy(ones_lhs_bf[:], ones_lhs_s[:])
    sel2 = const.tile([2, 128], F32R)
    nc.sync.dma_start(sel2[:], io["sel"])
    sel2b = const.tile([2, 128], BF16, tag="sel2b")
    nc.vector.tensor_copy(sel2b[:], sel2[:].bitcast(F32))
    ones_row = const.tile([1, 128], F32, tag="ones_row")  # K=1 bcast lhsT
    nc.vector.memset(ones_row[:], 1.0)
    aux_sb = const.tile([128, 48], F32)
    nc.sync.dma_start(aux_sb[:], io["aux"])
    eps_t = const.tile([1, 1], F32)
    nc.vector.memset(eps_t[:], EPS)
    masks = []
    for i in range(CH // KB):
        mt = const.tile([128, 2, KB], BF16, tag=f"mask{i}")
        nc.vector.memset(mt[:], 1.0)
        nc.gpsimd.affine_select(mt[:], mt[:], pattern=[[0, 2], [1, KB]],
                                compare_op=ALU.is_ge, fill=0.0, base=0,
                                channel_multiplier=-1)
        masks.append(mt)

    # ---------------- phase 1: LN1 + q,k,v + residual staging ----------------
    bigX = tc.alloc_tile_pool(name="bigX", bufs=1, side="right")    # residual (thru end)
    bigV = tc.alloc_tile_pool(name="bigV", bufs=1, side="right")    # v (thru attn)
    bigQK = tc.alloc_tile_pool(name="bigQK", bufs=1, side="right")  # q,k (thru attn)
    bigA = tc.alloc_tile_pool(name="bigA", bufs=1)                  # xs (phase 1 only)
    wvp = tc.alloc_tile_pool(name="wvp", bufs=1)
    wqkp = tc.alloc_tile_pool(name="wqkp", bufs=1)
    ph1_in = tc.alloc_tile_pool(name="ph1_in", bufs=2)
    ph1_sq = tc.alloc_tile_pool(name="ph1_sq", bufs=3)
    ph1_ps = tc.alloc_tile_pool(name="ph1_ps", bufs=1, space="PSUM")
    ph1_st = tc.alloc_tile_pool(name="ph1_st", bufs=1)
    v_ps = tc.alloc_tile_pool(name="v_ps", bufs=2, space="PSUM")
    qk_ps = tc.alloc_tile_pool(name="qk_ps", bufs=2, space="PSUM")
    xr_in = tc.alloc_tile_pool(name="xr_in", bufs=1)

    xs = [bigA.tile([128, T], BF16, tag=f"xs{k}", name=f"xs{k}") for k in range(KT_C)]
    xs_tail = bigA.tile([128, T], BF16, tag="xs_tail")   # row0 = mu', row1 = ones
    nc.vector.memset(xs_tail[:], 0)
    nc.vector.tensor_copy(xs_tail[0:2, :], ones_stage[:])
    v_sb = [bigV.tile([128, HL, D + 1], BF16, tag=f"v{tb}", name=f"v{tb}")
            for tb in range(T // 128)]
    qT = [bigQK.tile([128, T], BF16, tag=f"qT{hp}", name=f"qT{hp}")
          for hp in range(HL // 2)]
    kT = [bigQK.tile([128, T], BF16, tag=f"kT{hp}", name=f"kT{hp}")
          for hp in range(HL // 2)]
    x2 = [bigX.tile([128, TO], BF16, tag=f"x2_{k}", name=f"x2_{k}")
          for k in range(KT_C)]

    # prefetch chunk 0 of x first: the LN1 stats chain is the critical path
    # at kernel start, the weights below overlap it on the scalar DMA queue
    xin0 = []
    for k in range(KT_C):
        xt = ph1_in.tile([128, CH], F32R, tag=f"xin{k}", name=f"xin0_{k}")
        nc.sync.dma_start(xt[:], xT[k * 128:(k + 1) * 128, 0:CH])
        xin0.append(xt)

    wv_t = wvp.tile([128, KS, HL * (D + 1)], BF16, tag="wv_main")
    nc.scalar.dma_start(wv_t[:], wv.rearrange("(kt p) f -> p kt f", p=128))
    wqk_t = []
    for hp in range(HL // 2):
        pair = []
        for kind, w_ap in enumerate((wq, wk)):
            wtile = wqkp.tile([128, KS, 128], BF16, tag=f"wqk{hp}_{kind}")
            nc.scalar.dma_start(
                wtile[:],
                w_ap[:, hp * 128:(hp + 1) * 128]
                .rearrange("(kt p) f -> p kt f", p=128))
            pair.append(wtile)
        wqk_t.append(pair)

    def qk_chunk(c):
        sl = slice(c * CH, (c + 1) * CH)
        for hp in range(HL // 2):
            for kind in range(2):
                qp = qk_ps.tile([128, CH], F32, tag="qk_ps",
                                name=f"qkps{hp}_{kind}_{c}")
                for k in range(KS):
                    rhs = xs[k][:, sl] if k < KT_C else xs_tail[:, sl]
                    nc.tensor.matmul(qp[:], _r(wqk_t[hp][kind][:, k, :]), _r(rhs),
                                     start=(k == 0), stop=(k == KS - 1))
                dest = qT[hp] if kind == 0 else kT[hp]
                nc.vector.tensor_copy(dest[:, sl], qp[:])

    for c in range(NCH):
        sl = slice(c * CH, (c + 1) * CH)
        xbs = []
        # 1) x chunk DMA + bf16 staging casts first (DVE queue head)
        for k in range(KT_C):
            if c == 0:
                xt = xin0[k]
            else:
                xt = ph1_in.tile([128, CH], F32R, tag=f"xin{k}",
                                 name=f"xin{c}_{k}")
                nc.sync.dma_start(xt[:], xT[k * 128:(k + 1) * 128, sl])
            # stats run on a bf16 copy staged straight into xs (normalized
            # in place later): bf16 matmuls stream ~2x faster than fp32 and
            # the LN stats lose <0.1% accuracy from the rounding
            xb = xs[k][:, sl]
            nc.vector.tensor_copy(xb, xt[:])
            xbs.append(xb)
        # 3) LN1 stats + normalize
        sum_ps = ph1_ps.tile([1, CH], F32, tag="sum", name=f"sum{c}")
        sq_ps = ph1_ps.tile([1, CH], F32, tag="sq", name=f"sq{c}")
        for k in range(KT_C):
            x2q = ph1_sq.tile([128, CH], BF16, tag="x2", name=f"x2_{c}_{k}")
            nc.vector.tensor_mul(x2q[:], xbs[k], xbs[k])
            nc.tensor.matmul(sum_ps[:], ones_lhs_bf[:], xbs[k],
                             start=(k == 0), stop=(k == KT_C - 1))
            nc.tensor.matmul(sq_ps[:], ones_lhs_bf[:], x2q[:],
                             start=(k == 0), stop=(k == KT_C - 1))
        mu = ph1_st.tile([1, CH], F32, tag="mu", name=f"mu{c}")
        nc.scalar.mul(mu[:], sum_ps[:], 1.0 / C)
        var = ph1_st.tile([1, CH], F32, tag="var", name=f"var{c}")
        nc.scalar.mul(var[:], sq_ps[:], 1.0 / C)
        mu2 = ph1_st.tile([1, CH], F32, tag="mu2", name=f"mu2_{c}")
        nc.vector.tensor_mul(mu2[:], mu[:], mu[:])
        nc.vector.tensor_tensor(out=var[:], in0=var[:], in1=mu2[:], op=ALU.subtract)
        rstd = ph1_st.tile([1, CH], F32, tag="rstd", name=f"rstd{c}")
        nc.scalar.activation(rstd[:], var[:], AF.Sqrt, bias=eps_t[:])
        nc.vector.reciprocal(rstd[:], rstd[:])
        nc.vector.tensor_mul(xs_tail[0:1, sl], mu[:], rstd[:])
        rstd_bf = ph1_st.tile([1, CH], BF16, tag="rstd_bf", name=f"rstd_bf{c}")
        nc.vector.tensor_copy(rstd_bf[:], rstd[:])
        rstd_b = ph1_st.tile([128, CH], BF16, tag="rstd_b", name=f"rstd_b{c}")
        nc.gpsimd.partition_broadcast(rstd_b[:], rstd_bf[:])
        for k in range(KT_C):
            nc.vector.tensor_mul(xs[k][:, sl], xbs[k], rstd_b[:])
        # qk for the previous chunk fills the PE while this chunk's stats drain
        if c > 0:
            qk_chunk(c - 1)
        # v for this chunk's token blocks: v_sb[tb] = [tok, head, d | ones]
        for tb in range(c * CH // 128, (c + 1) * CH // 128):
            tsl = slice(tb * 128, (tb + 1) * 128)
            for half in range(2):
                fsl = slice(half * (HL // 2) * (D + 1),
                            (half + 1) * (HL // 2) * (D + 1))
                vp = v_ps.tile([128, (HL // 2) * (D + 1)], F32, tag="v_ps",
                               name=f"vps{tb}_{half}")
                for k in range(KS):
                    lhs = xs[k][:, tsl] if k < KT_C else xs_tail[:, tsl]
                    nc.tensor.matmul(vp[:], _r(lhs), _r(wv_t[:, k, fsl]),
                                     start=(k == 0), stop=(k == KS - 1))
                nc.vector.tensor_copy(
                    v_sb[tb][:, half * (HL // 2):(half + 1) * (HL // 2), :], vp[:])
    qk_chunk(NCH - 1)

    # residual staging: x2[k] = bf16(xres[k] + b_proj[k]) (consumed in phase 3)
    for k in range(KT_C):
        xr = xr_in.tile([128, TO], F32, tag="xr", name=f"xr{k}")
        nc.sync.dma_start(xr[:], xres[k * 128:(k + 1) * 128, :])
        nc.vector.tensor_scalar_add(x2[k][:], xr[:], aux_sb[:, k:k + 1])

    for p in (xr_in, qk_ps, v_ps, ph1_st, ph1_ps, ph1_sq, ph1_in,
              wqkp, wvp, bigA):
        p.release()

    # ---------------- phase 2: attention + proj + split ReduceScatter ----------
    fc_w = tc.alloc_tile_pool(name="fc_w", bufs=14)    # fc1 weights; outlives
    bigY = tc.alloc_tile_pool(name="bigY", bufs=1)     # yT (lives thru proj)
    wpp = tc.alloc_tile_pool(name="wpp", bufs=1)
    sc_ps = tc.alloc_tile_pool(name="sc_ps", bufs=2, space="PSUM")
    ex_sb = tc.alloc_tile_pool(name="ex_sb", bufs=3)
    av_ps = tc.alloc_tile_pool(name="av_ps", bufs=1, space="PSUM")
    bc_ps = tc.alloc_tile_pool(name="bc_ps", bufs=1, space="PSUM")
    yn_sb = tc.alloc_tile_pool(name="yn_sb", bufs=2)
    pr_ps = tc.alloc_tile_pool(name="pr_ps", bufs=1, space="PSUM")
    pr_sb = tc.alloc_tile_pool(name="pr_sb", bufs=4)
    dram = tc.alloc_tile_pool(name="dram", bufs=1, space="DRAM")

    yT = [bigY.tile([128, T], BF16, tag=f"yT{k}", name=f"yT{k}")
          for k in range(HL * D // 128)]

    wp_t = wpp.tile([128, KT_P, C], BF16, tag="wp_main")
    nc.sync.dma_start(wp_t[:], wp.rearrange("(kt p) f -> p kt f", p=128))

    # rs_in[m] slot0 = chunk m partial, slot1 = chunk m+2 partial; the
    # pairwise ReduceScatter hands chunk m to the even core, m+2 to the odd.
    rs_in = [dram.tile([2, C, CH], BF16, tag=f"rsin{m}", name=f"rsin{m}")
             for m in range(2)]
    rs_out = [dram.tile([C, CH], BF16, tag=f"rsout{m}", name=f"rsout{m}")
              for m in range(2)]

    def fc1_wt(m, f):
        wt = fc_w.tile([128, KT_C, 128], BF16, tag="wfc_t", name=f"wfc{m}_{f}")
        nc.sync.dma_start(
            wt[:],
            wfc[:, f * 128:(f + 1) * 128].rearrange("(kt p) n -> p kt n", p=128))
        return wt

    fc1_pre = []

    # Schraudolph fast-exp constants for bf16: bitcast(int16(A*x + B)),
    # A folds the 1/sqrt(D) score scale; max rel err ~3.3%, which the softmax
    # row-normalization largely cancels (<=0.006 abs on attention out).
    SCHR_A = 184.664965 / np.sqrt(D)
    SCHR_B = 16250.4

    # chunk order (1,3,2,0): rs pair 1 (chunks 1,3) is issued at 60% of the
    # attention work and its transfer lands inside long chunk 2, where the
    # deferred normalize absorbs the collective's DMA freeze; rs pair 0
    # (chunks 0,2) goes at the end, hidden under phase 3's first half.
    for qc in (1, 3, 0, 2):
        qsl = slice(qc * CH, (qc + 1) * CH)
        n_kb = (qc + 1) * (CH // KB)      # diag blocks are the last 4
        if qc == 2:
            # prefetch fc1 weights for token-half 1 during the last chunk's
            # attention: regular DMA freezes while the final collective runs,
            # and these 14 tiles (~27us of matmul) keep the PE fed through it
            for f in range(14):
                fc1_pre.append(fc1_wt(1, f))
        norm = []                         # per-hp deferred normalize state
        for hp in range(HL // 2):
            yps = [av_ps.tile([128, CH], F32, tag=f"av{hh}",
                              name=f"av{hp}_{hh}_{qc}") for hh in range(2)]
            for kb in range(n_kb):
                diag_j = kb - (n_kb - CH // KB)
                q0 = max(0, diag_j * KB)
                ksl = slice(kb * KB, (kb + 1) * KB)
                # both heads' scores into one 2-bank PSUM tile, one batched exp
                sp = sc_ps.tile([128, 2, CH], F32, tag="sc",
                                name=f"sc{hp}_{qc}_{kb}")
                for hh in range(2):
                    prow = slice(hh * D, (hh + 1) * D)
                    nc.tensor.matmul(sp[:, hh, q0:], kT[hp][prow, ksl],
                                     qT[hp][prow, qsl][:, q0:],
                                     start=True, stop=True)
                if diag_j < 0 and kb % 3 == 2:
                    # off-diagonal block: Schraudolph exp on the DVE to
                    # unload the ACT engine (the attention pacer)
                    eti = ex_sb.tile([128, 2, CH], mybir.dt.int16, tag="exi",
                                     name=f"exi{hp}_{qc}_{kb}")
                    with nc.allow_low_precision(reason="schraudolph exp"):
                        nc.vector.tensor_scalar(
                            out=eti[:], in0=sp[:], scalar1=SCHR_A,
                            scalar2=SCHR_B, op0=ALU.mult, op1=ALU.add)
                    et_h = [eti[:, hh, :].bitcast(BF16) for hh in range(2)]
                else:
                    et = ex_sb.tile([128, 2, CH], BF16, tag="ex",
                                    name=f"ex{hp}_{qc}_{kb}")
                    nc.scalar.activation(et[:, :, q0:], sp[:, :, q0:],
                                         AF.Exp, scale=1.0 / np.sqrt(D))
                    if diag_j >= 0:
                        # causal mask on DVE (gpsimd is busy with the chunk
                        # ReduceScatter; affine_select there stalls AVs)
                        nc.vector.tensor_mul(
                            et[:, :, q0:q0 + KB], et[:, :, q0:q0 + KB],
                            masks[q0 // KB][:])
                    et_h = [et[:, hh, q0:] for hh in range(2)]
                for hh in range(2):
                    nc.tensor.matmul(yps[hh][:D + 1, q0:],
                                     v_sb[kb][:, hp * 2 + hh, :],
                                     et_h[hh],
                                     start=(kb == 0), stop=(kb == n_kb - 1))
            # stage y and issue the 1/rowsum chain now; the PE-side broadcast
            # and final muls run after the hp loop so the in-order PE queue
            # never parks behind this DVE/DMA latency chain
            ycop = [yn_sb.tile([D + 1, CH], BF16, tag=f"ycop{hh}_{hp}",
                               name=f"yc{hp}_{hh}_{qc}") for hh in range(2)]
            for hh in range(2):
                nc.vector.tensor_copy(ycop[hh][:], yps[hh][:D + 1, :])
            ypair_hi = yn_sb.tile([128, CH], BF16, tag=f"ypair_{hp}",
                                  name=f"yp{hp}_{qc}")
            nc.sync.dma_start(ypair_hi[D:2 * D, :], ycop[1][:D, :])
            # 1/rowsum: scatter each [1,512] sum row over 128 partitions so
            # the iterative-divide reciprocal runs at FD=8, then gather back.
            srp = yn_sb.tile([128, 2, CH // 128], BF16, tag=f"srp_{hp}",
                             name=f"srp{hp}_{qc}")
            srec = yn_sb.tile([2, CH], BF16, tag=f"srec_{hp}",
                              name=f"sr{hp}_{qc}")
            for hh in range(2):
                nc.sync.dma_start(srp[:, hh, :], ycop[hh][D:D + 1, :])
            with nc.allow_low_precision(reason="1/rowsum rounded to bf16"):
                nc.vector.reciprocal(srp[:], srp[:])
            for hh in range(2):
                nc.sync.dma_start(srec[hh:hh + 1, :], srp[:, hh, :])
            norm.append((ycop, ypair_hi, srec))
        for hp, (ycop, ypair_hi, srec) in enumerate(norm):
            bc = bc_ps.tile([128, CH], F32, tag="bc", name=f"bc{hp}_{qc}")
            nc.tensor.matmul(bc[:], sel2b[:], srec[:], start=True, stop=True)
            nc.vector.tensor_mul(yT[hp][0:D, qsl], ycop[0][:D, :], bc[0:D, :])
            nc.vector.tensor_mul(yT[hp][D:2 * D, qsl], ypair_hi[D:2 * D, :],
                                 bc[D:2 * D, :])
        # proj partial for this chunk; collective inputs go via the gpsimd
        # DMA queue so the Sync queue stays short for the softmax DMAs
        m, slot = qc % 2, qc // 2
        for ob in range(C // 128):
            osl = slice(ob * 128, (ob + 1) * 128)
            pp = pr_ps.tile([128, CH], F32, tag="pr", name=f"pr{qc}_{ob}")
            for k in range(KT_P):
                nc.tensor.matmul(pp[:], _r(wp_t[:, k, osl]), yT[k][:, qsl],
                                 start=(k == 0), stop=(k == KT_P - 1))
            pt = pr_sb.tile([128, CH], BF16, tag="pr_sb", name=f"prs{qc}_{ob}")
            nc.vector.tensor_copy(pt[:], pp[:])
            nc.gpsimd.dma_start(rs_in[m][slot, osl, :], pt[:])
        if qc >= 2:
            nc.gpsimd.collective_compute(
                "ReduceScatter", ALU.add,
                replica_groups=[[0, 1], [2, 3], [4, 5], [6, 7]],
                ins=[rs_in[m].opt()], outs=[rs_out[m].opt()])

    for p in (pr_sb, pr_ps, yn_sb, bc_ps, av_ps, ex_sb, sc_ps, wpp, bigY,
              bigQK, bigV):
        p.release()

    # ---------------- phase 3: residual + LN2 + MLP ----------------
    fc_ps = tc.alloc_tile_pool(name="fc_ps", bufs=3, space="PSUM")
    h_pool = tc.alloc_tile_pool(name="h_pool", bufs=1)
    out_sb = tc.alloc_tile_pool(name="out_sb", bufs=3)
    res_in = tc.alloc_tile_pool(name="res_in", bufs=2)
    ln2_ps = tc.alloc_tile_pool(name="ln2_ps", bufs=1, space="PSUM")
    ln2_sq = tc.alloc_tile_pool(name="ln2_sq", bufs=2)
    ln2_st = tc.alloc_tile_pool(name="ln2_st", bufs=1)

    xs2 = [bigX.tile([128, TO], BF16, tag=f"xs2_{k}", name=f"xs2_{k}")
           for k in range(KT_C)]
    hT = [[h_pool.tile([128, CH], BF16, tag=f"h{m}_{f}", name=f"hT{m}_{f}")
           for f in range(KT_F)] for m in range(2)]

    def ln2_stats(m):
        # residual add + LN2 stats; residual reads go on the Sync queue (the
        # gpsimd queue parks behind the pending ReduceScatter triggers)
        msl = slice(m * CH, (m + 1) * CH)
        for k in range(KT_C):
            rt = res_in.tile([128, CH], BF16, tag="rs_t", name=f"rst{m}_{k}")
            nc.sync.dma_start(rt[:], rs_out[m][k * 128:(k + 1) * 128, :])
            nc.vector.tensor_add(x2[k][:, msl], x2[k][:, msl], rt[:])
        sum_ps = ln2_ps.tile([1, CH], F32, tag="sum", name=f"l2sum{m}")
        sq_ps = ln2_ps.tile([1, CH], F32, tag="sq", name=f"l2sq{m}")
        for k in range(KT_C):
            x2q = ln2_sq.tile([128, CH], BF16, tag="x2", name=f"l2x2_{m}_{k}")
            nc.vector.tensor_mul(x2q[:], x2[k][:, msl], x2[k][:, msl])
            nc.tensor.matmul(sum_ps[:], ones_lhs_bf[:], x2[k][:, msl],
                             start=(k == 0), stop=(k == KT_C - 1))
            nc.tensor.matmul(sq_ps[:], ones_lhs_bf[:], x2q[:],
                             start=(k == 0), stop=(k == KT_C - 1))
        return sum_ps, sq_ps

    def ln2_norm(m, sum_ps, sq_ps):
        # broadcasts run as K=2 matmuls (the gpsimd partition_broadcast
        # would park behind the pending ReduceScatter trigger on that queue)
        msl = slice(m * CH, (m + 1) * CH)
        mu = ln2_st.tile([1, CH], F32, tag=f"mu{m}", name=f"l2mu{m}")
        nc.scalar.mul(mu[:], sum_ps[:], 1.0 / C)
        var = ln2_st.tile([1, CH], F32, tag=f"var{m}", name=f"l2var{m}")
        nc.scalar.mul(var[:], sq_ps[:], 1.0 / C)
        mu2 = ln2_st.tile([1, CH], F32, tag=f"mu2_{m}", name=f"l2mu2_{m}")
        nc.vector.tensor_mul(mu2[:], mu[:], mu[:])
        nc.vector.tensor_tensor(out=var[:], in0=var[:], in1=mu2[:], op=ALU.subtract)
        rstd = ln2_st.tile([1, CH], F32, tag=f"rstd{m}", name=f"l2rstd{m}")
        nc.scalar.activation(rstd[:], var[:], AF.Sqrt, bias=eps_t[:])
        nc.vector.reciprocal(rstd[:], rstd[:])
        nmr = ln2_st.tile([1, CH], F32, tag=f"nmr{m}", name=f"l2nmr{m}")
        nc.vector.tensor_mul(nmr[:], mu[:], rstd[:])
        rstd_b = ln2_ps.tile([128, CH], F32, tag="rb", name=f"l2rb{m}")
        nc.tensor.matmul(rstd_b[:], ones_row[:], rstd[:],
                         start=True, stop=True)
        nmr_b = ln2_ps.tile([128, CH], F32, tag="nb", name=f"l2nb{m}")
        nc.tensor.matmul(nmr_b[:], ones_row[:], nmr[:],
                         start=True, stop=True)
        for k in range(KT_C):
            tt = ln2_sq.tile([128, CH], BF16, tag="tt", name=f"l2t{m}_{k}")
            nc.vector.tensor_mul(tt[:], x2[k][:, msl], rstd_b[:])
            nc.vector.tensor_tensor(out=xs2[k][:, msl], in0=tt[:],
                                    in1=nmr_b[:], op=ALU.subtract)

    def fc1_chunk(m, pre=(), mid_hook=None):
        msl = slice(m * CH, (m + 1) * CH)
        for f in range(KT_F):
            if f == KT_F - 6 and mid_hook is not None:
                # other chunk's LN2 stats interleave into this chunk's tail
                # so its scalar/DVE chain hides under the last f-tiles
                mid_hook()
            wt = pre[f] if f < len(pre) else fc1_wt(m, f)
            hps = fc_ps.tile([128, CH], F32, tag="fc1", name=f"fc1p{m}_{f}")
            for k in range(KT_C):
                nc.tensor.matmul(hps[:], _r(wt[:, k, :]), xs2[k][:, msl],
                                 start=(k == 0), stop=(k == KT_C - 1))
            nc.scalar.activation(hT[m][f][:], hps[:], AF.Gelu_apprx_tanh,
                                 bias=aux_sb[:, 8 + f:9 + f])

    # token-half 1 first: its ReduceScatter was issued at 60% of attention,
    # so it is complete by now; half 0's collective hides under fc1_chunk(1)
    st1 = ln2_stats(1)
    ln2_norm(1, *st1)
    st0 = []

    def _stats0():
        st0.append(ln2_stats(0))

    fc1_chunk(1, pre=fc1_pre, mid_hook=_stats0)
    ln2_norm(0, *st0[0])
    fc1_chunk(0)
    for p in (ln2_st, ln2_sq, ln2_ps, res_in):
        p.release()

    fc2_w = tc.alloc_tile_pool(name="fc2_w", bufs=3)
    fc2_ps = tc.alloc_tile_pool(name="fc2_ps", bufs=2, space="PSUM")
    n_fg = (KT_F2 + FG - 1) // FG
    for ob in range(C // 128):
        osl = slice(ob * 128, (ob + 1) * 128)
        ops = [fc2_ps.tile([128, CH], F32, tag=f"fc2m{m}", name=f"fc2p{m}_{ob}")
               for m in range(2)]
        for fg in range(n_fg):
            lo, hi = fg * FG, min(fg * FG + FG, KT_F2)
            w2 = fc2_w.tile([128, FG, 128], BF16, tag="wfc2_t",
                            name=f"w2_{ob}_{fg}")
            nc.sync.dma_start(
                w2[:, : hi - lo, :],
                wfc2[lo * 128: hi * 128, osl]
                .rearrange("(kt p) n -> p kt n", p=128))
            # j-outer / m-inner: consecutive matmuls share the stationary
            # weight tile, halving the effective LDWEIGHTS pressure
            for j in range(lo, hi):
                for m in range(2):
                    nc.tensor.matmul(ops[m][:], _r(w2[:, j - lo, :]),
                                     _r(hT[m][j][:]),
                                     start=(j == 0), stop=(j == KT_F2 - 1))
        for m in range(2):
            msl = slice(m * CH, (m + 1) * CH)
            ot = out_sb.tile([128, CH], F32, tag="ot", name=f"ot{m}_{ob}")
            nc.vector.scalar_tensor_tensor(
                out=ot[:], in0=ops[m][:], scalar=aux_sb[:, 40 + ob:41 + ob],
                in1=x2[ob][:, msl], op0=ALU.add, op1=ALU.add)
            nc.sync.dma_start(out_ap[osl, msl], ot[:])

    for p in (fc2_ps, fc2_w, out_sb, h_pool, fc_ps, fc_w, dram, bigX, const):
        p.release()


_NC_CACHE = None


def _build():
    global _NC_CACHE
    if _NC_CACHE is not None:
        return _NC_CACHE
    nc = bacc.Bacc("TRN2", target_bir_lowering=False, debug=False, num_devices=8)
    io = {
        "xT": nc.dram_tensor("xT", [C, T], F32R, kind="ExternalInput").ap(),
        "xres": nc.dram_tensor("xres", [C, TO], F32, kind="ExternalInput").ap(),
        "wq": nc.dram_tensor("wq", [CP, HL * D], BF16, kind="ExternalInput").ap(),
        "wk": nc.dram_tensor("wk", [CP, HL * D], BF16, kind="ExternalInput").ap(),
        "wv": nc.dram_tensor("wv", [CP, HL * (D + 1)], BF16,
                             kind="ExternalInput").ap(),
        "wp": nc.dram_tensor("wp", [HL * D, C], BF16, kind="ExternalInput").ap(),
        "wfc": nc.dram_tensor("wfc", [C, FF], BF16, kind="ExternalInput").ap(),
        "wfc2": nc.dram_tensor("wfc2", [FP2, C], BF16, kind="ExternalInput").ap(),
        "sel": nc.dram_tensor("sel", [2, 128], F32R, kind="ExternalInput").ap(),
        "aux": nc.dram_tensor("aux", [128, 48], F32, kind="ExternalInput").ap(),
        "out": nc.dram_tensor("out", [C, TO], F32, kind="ExternalOutput").ap(),
    }
    with tile.TileContext(nc) as tc:
        _emit(tc, io)
    nc.compile()
    _NC_CACHE = nc
    return nc


def _stack_ln(w, g, b, bias):
    """[w*g ; -colsum(w*g) ; b@w + bias ; zero pad] -> [CP, F] float32."""
    wg = (w * g[:, None]).astype(np.float32)
    out = np.zeros((CP, w.shape[1]), np.float32)
    out[:C] = wg
    out[C] = -wg.sum(0)
    out[C + 1] = b @ w + bias
    return out


def kernel(x, ln1_g, ln1_b, w_attn, b_attn, w_proj, b_proj,
           ln2_g, ln2_b, w_fc, b_fc, w_fc2, b_fc2):
    f32 = lambda a: np.asarray(a, np.float32)
    x = f32(x)
    ln1_g, ln1_b, w_attn, b_attn = f32(ln1_g), f32(ln1_b), f32(w_attn), f32(b_attn)
    w_proj, b_proj, ln2_g, ln2_b = f32(w_proj), f32(b_proj), f32(ln2_g), f32(ln2_b)
    w_fc, b_fc, w_fc2, b_fc2 = f32(w_fc), f32(b_fc), f32(w_fc2), f32(b_fc2)

    nc = _build()

    qkv_stack = _stack_ln(w_attn, ln1_g, ln1_b, b_attn)        # [C+2, 3C]
    fc_w_g = (w_fc * ln2_g[:, None]).astype(np.float32)        # [C, FF]
    bfc_eff = ln2_b @ w_fc + b_fc                              # [FF]
    fc2_stack = w_fc2.astype(np.float32)                       # [FF, C]

    sel_np = np.zeros((2, 128), np.float32)
    sel_np[0, :64] = 1.0
    sel_np[1, 64:] = 1.0

    aux_np = np.zeros((128, 48), np.float32)
    aux_np[:, :KT_C] = b_proj.reshape(KT_C, 128).T             # x2 staging bias
    aux_np[:, 8:8 + KT_F] = bfc_eff.reshape(KT_F, 128).T       # gelu bias
    aux_np[:, 40:48] = b_fc2.reshape(KT_C, 128).T              # fc2 output bias

    in_maps = []
    for core in range(8):
        b_idx, r = core // 2, core % 2
        hsl = slice(r * HL * D, (r + 1) * HL * D)              # this core's heads
        xT_b = np.ascontiguousarray(x[b_idx].T)                # [C, T]
        wv_cols = qkv_stack[:, 2 * C + r * HL * D: 2 * C + (r + 1) * HL * D]
        wv_aug = np.zeros((CP, HL, D + 1), np.float32)
        wv_aug[:, :, :D] = wv_cols.reshape(CP, HL, D)
        wv_aug[C + 1, :, D] = 1.0                              # ones col via ones-row
        wp_loc = np.ascontiguousarray(w_proj[r * HL * D:(r + 1) * HL * D, :])
        bf = ml_dtypes.bfloat16
        in_maps.append({
            "xT": xT_b,
            "xres": np.ascontiguousarray(xT_b[:, r * TO:(r + 1) * TO]),
            "wq": np.ascontiguousarray(qkv_stack[:, hsl]).astype(bf),
            "wk": np.ascontiguousarray(
                qkv_stack[:, C + r * HL * D: C + (r + 1) * HL * D]).astype(bf),
            "wv": np.ascontiguousarray(
                wv_aug.reshape(CP, HL * (D + 1))).astype(bf),
            "wp": wp_loc.astype(bf),
            "wfc": fc_w_g.astype(bf),
            "wfc2": fc2_stack.astype(bf),
            "sel": sel_np,
            "aux": aux_np,
        })

    trace = bool(int(os.environ.get("KERNEL_TRACE", "0")))
    res = run_bass_kernel_spmd(nc, in_maps, core_ids=list(range(8)), trace=trace)
    kernel.last_result = res

    out = np.empty((B, T, C), np.float32)
    for core in range(8):
        b_idx, r = core // 2, core % 2
        out[b_idx, r * TO:(r + 1) * TO, :] = res.results[core]["out"].T
    return out


# revision 60
# speedup vs baseline: 1.0608x; 1.0608x over previous
"""Trainium2 Bass kernel for a dense transformer block (LN-attn-LN-MLP).

Sharding: core pair (2b, 2b+1) handles batch b. Each core computes 8 of the
16 attention heads over the full 2048-token sequence (head/tensor parallel),
then the pair ReduceScatters the partial c_proj output over tokens; the MLP
runs on each core's 1024-token half. All activations on chip are kept
feature-major [features, tokens] so no transposes are needed; the host
pre-transposes x and post-transposes the outputs.

Pipeline structure (v4):
 - Phase 1 prefetches the first x chunk ahead of the weights, computes LN1
   stats, q, k and v for all local heads, and stages the residual (bf16,
   with b_proj folded in) while the PE has slack.
 - Phase 2 runs attention query-chunk-major in order (1,3,0,2) so each pair
   ReduceScatter is issued as early as its two chunks allow; per chunk both
   heads of a pair share one PSUM score tile and a single batched exp
   ACTIVATE; ~1/3 of the off-diagonal exp blocks run on the DVE instead via
   a Schraudolph bit-trick exp (int16 bits -> bf16 bitcast) to unload the
   ACT engine that otherwise paces attention. The softmax 1/rowsum chains
   for all four head pairs are software-pipelined: reciprocal chains issue
   inside the head loop, the PE-side broadcast matmuls and the final muls
   run batched at chunk end so the in-order PE queue never waits on them.
   proj partials are written to the collective buffers on the gpsimd queue
   (keeps the Sync DMA queue short for the tiny softmax DMAs).
 - Phase 3 processes token-half 1 then 0 (matching collective completion
   order), adds the residual, runs LN2 (mean subtraction on DVE; gamma in
   the weights, beta/bias on the GELU activation bias) and the MLP per
   512-token chunk; fc1 contracts over exactly 8 k-tiles.
"""

import os
import sys

sys.path.insert(0, "/opt/trn_rl_repo")

import ml_dtypes
import numpy as np

import concourse.bass as bass
import concourse.tile as tile
from concourse import bacc, mybir
from concourse.bass_utils import run_bass_kernel_spmd

F32 = mybir.dt.float32
F32R = mybir.dt.float32r
BF16 = mybir.dt.bfloat16
AF = mybir.ActivationFunctionType
ALU = mybir.AluOpType

# Model dims
C = 1024            # embed
T = 2048            # sequence length (full context per core)
B = 4               # batch
NHEAD = 16
D = 64              # head dim
HL = 8              # local heads per core
TO = 1024           # output tokens per core (after pair ReduceScatter)
CH = 512            # token chunk (matmul free dim)
NCH = T // CH       # 4
KB = 128            # key block
FF = 4 * C          # 4096
EPS = 1e-5

KT_C = C // 128     # 8 k-tiles over embed dim
KS = KT_C + 1       # k-tiles incl. the mu'/ones/zero-pad tail tile (qkv only)
CP = KS * 128       # padded stack rows (1152)
KT_P = HL * D // 128       # proj k-tiles (4; bias folded into residual)
KT_F = FF // 128           # 32
KT_F2 = FF // 128 + 1      # fc2 k-tiles incl. tail (33)
FP2 = KT_F2 * 128          # padded fc2 rows (4224)
FG = 8                     # fc2 weight k-tiles per DMA group


def _r(ap):
    return ap.bitcast(F32R) if ap.dtype == F32 else ap


def _emit(tc, io):
    nc = tc.nc
    xT, xres, wq, wk, wv, wp, wfc, wfc2, out_ap = (
        io["xT"], io["xres"], io["wq"], io["wk"], io["wv"], io["wp"],
        io["wfc"], io["wfc2"], io["out"])

    const = tc.alloc_tile_pool(name="const", bufs=1)
    ones_stage = const.tile([2, T], F32)
    nc.vector.memset(ones_stage[:], 1.0)
    ones_lhs_s = const.tile([128, 1], F32)
    nc.vector.memset(ones_lhs_s[:], 1.0)
    ones_lhs = const.tile([128, 1], F32R)
    nc.vector.tensor_copy(ones_lhs[:], ones_lhs_s[:])
    ones_lhs_bf = const.tile([128, 1], BF16)
    nc.vector.tensor_copy(ones_lhs_bf[:], ones_lhs_s[:])
    sel2 = const.tile([2, 128], F32R)
    nc.sync.dma_start(sel2[:], io["sel"])
    sel2b = const.tile([2, 128], BF16, tag="sel2b")
    nc.vector.tensor_copy(sel2b[:], sel2[:].bitcast(F32))
    ones_row = const.tile([1, 128], F32, tag="ones_row")  # K=1 bcast lhsT
    nc.vector.memset(ones_row[:], 1.0)
    aux_sb = const.tile([128, 40], F32)
    nc.sync.dma_start(aux_sb[:], io["aux"])
    eps_t = const.tile([1, 1], F32)
    nc.vector.memset(eps_t[:], EPS)
    masks = []
    for i in range(CH // KB):
        mt = const.tile([128, 2, KB], BF16, tag=f"mask{i}")
        nc.vector.memset(mt[:], 1.0)
        nc.gpsimd.affine_select(mt[:], mt[:], pattern=[[0, 2], [1, KB]],
                                compare_op=ALU.is_ge, fill=0.0, base=0,
                                channel_multiplier=-1)
        masks.append(mt)

    # ---------------- phase 1: LN1 + q,k,v + residual staging ----------------
    bigX = tc.alloc_tile_pool(name="bigX", bufs=1, side="right")    # residual (thru end)
    bigV = tc.alloc_tile_pool(name="bigV", bufs=1, side="right")    # v (thru attn)
    bigQK = tc.alloc_tile_pool(name="bigQK", bufs=1, side="right")  # q,k (thru attn)
    bigA = tc.alloc_tile_pool(name="bigA", bufs=1)                  # xs (phase 1 only)
    wvp = tc.alloc_tile_pool(name="wvp", bufs=1)
    wqkp = tc.alloc_tile_pool(name="wqkp", bufs=1)
    ph1_in = tc.alloc_tile_pool(name="ph1_in", bufs=2)
    ph1_sq = tc.alloc_tile_pool(name="ph1_sq", bufs=3)
    ph1_ps = tc.alloc_tile_pool(name="ph1_ps", bufs=1, space="PSUM")
    ph1_st = tc.alloc_tile_pool(name="ph1_st", bufs=1)
    v_ps = tc.alloc_tile_pool(name="v_ps", bufs=2, space="PSUM")
    qk_ps = tc.alloc_tile_pool(name="qk_ps", bufs=2, space="PSUM")
    xr_in = tc.alloc_tile_pool(name="xr_in", bufs=1)

    xs = [bigA.tile([128, T], BF16, tag=f"xs{k}", name=f"xs{k}") for k in range(KT_C)]
    xs_tail = bigA.tile([128, T], BF16, tag="xs_tail")   # row0 = mu', row1 = ones
    nc.vector.memset(xs_tail[:], 0)
    nc.vector.tensor_copy(xs_tail[0:2, :], ones_stage[:])
    v_sb = [bigV.tile([128, HL, D + 1], BF16, tag=f"v{tb}", name=f"v{tb}")
            for tb in range(T // 128)]
    qT = [bigQK.tile([128, T], BF16, tag=f"qT{hp}", name=f"qT{hp}")
          for hp in range(HL // 2)]
    kT = [bigQK.tile([128, T], BF16, tag=f"kT{hp}", name=f"kT{hp}")
          for hp in range(HL // 2)]
    x2 = [bigX.tile([128, TO], BF16, tag=f"x2_{k}", name=f"x2_{k}")
          for k in range(KT_C)]

    # prefetch chunk 0 of x first: the LN1 stats chain is the critical path
    # at kernel start, the weights below overlap it on the scalar DMA queue
    xin0 = []
    for k in range(KT_C):
        xt = ph1_in.tile([128, CH], F32R, tag=f"xin{k}", name=f"xin0_{k}")
        nc.sync.dma_start(xt[:], xT[k * 128:(k + 1) * 128, 0:CH])
        xin0.append(xt)

    wv_t = wvp.tile([128, KS, HL * (D + 1)], BF16, tag="wv_main")
    nc.scalar.dma_start(wv_t[:], wv.rearrange("(kt p) f -> p kt f", p=128))
    wqk_t = []
    for hp in range(HL // 2):
        pair = []
        for kind, w_ap in enumerate((wq, wk)):
            wtile = wqkp.tile([128, KS, 128], BF16, tag=f"wqk{hp}_{kind}")
            nc.scalar.dma_start(
                wtile[:],
                w_ap[:, hp * 128:(hp + 1) * 128]
                .rearrange("(kt p) f -> p kt f", p=128))
            pair.append(wtile)
        wqk_t.append(pair)

    def qk_chunk(c):
        sl = slice(c * CH, (c + 1) * CH)
        for hp in range(HL // 2):
            for kind in range(2):
                qp = qk_ps.tile([128, CH], F32, tag="qk_ps",
                                name=f"qkps{hp}_{kind}_{c}")
                for k in range(KS):
                    rhs = xs[k][:, sl] if k < KT_C else xs_tail[:, sl]
                    nc.tensor.matmul(qp[:], _r(wqk_t[hp][kind][:, k, :]), _r(rhs),
                                     start=(k == 0), stop=(k == KS - 1))
                dest = qT[hp] if kind == 0 else kT[hp]
                nc.vector.tensor_copy(dest[:, sl], qp[:])

    for c in range(NCH):
        sl = slice(c * CH, (c + 1) * CH)
        xbs = []
        # 1) x chunk DMA + bf16 staging casts first (DVE queue head)
        for k in range(KT_C):
            if c == 0:
                xt = xin0[k]
            else:
                xt = ph1_in.tile([128, CH], F32R, tag=f"xin{k}",
                                 name=f"xin{c}_{k}")
                nc.sync.dma_start(xt[:], xT[k * 128:(k + 1) * 128, sl])
            # stats run on a bf16 copy staged straight into xs (normalized
            # in place later): bf16 matmuls stream ~2x faster than fp32 and
            # the LN stats lose <0.1% accuracy from the rounding
            xb = xs[k][:, sl]
            nc.vector.tensor_copy(xb, xt[:])
            xbs.append(xb)
        # 3) LN1 stats + normalize
        sum_ps = ph1_ps.tile([1, CH], F32, tag="sum", name=f"sum{c}")
        sq_ps = ph1_ps.tile([1, CH], F32, tag="sq", name=f"sq{c}")
        for k in range(KT_C):
            x2q = ph1_sq.tile([128, CH], BF16, tag="x2", name=f"x2_{c}_{k}")
            nc.vector.tensor_mul(x2q[:], xbs[k], xbs[k])
            nc.tensor.matmul(sum_ps[:], ones_lhs_bf[:], xbs[k],
                             start=(k == 0), stop=(k == KT_C - 1))
            nc.tensor.matmul(sq_ps[:], ones_lhs_bf[:], x2q[:],
                             start=(k == 0), stop=(k == KT_C - 1))
        mu = ph1_st.tile([1, CH], F32, tag="mu", name=f"mu{c}")
        nc.scalar.mul(mu[:], sum_ps[:], 1.0 / C)
        var = ph1_st.tile([1, CH], F32, tag="var", name=f"var{c}")
        nc.scalar.mul(var[:], sq_ps[:], 1.0 / C)
        mu2 = ph1_st.tile([1, CH], F32, tag="mu2", name=f"mu2_{c}")
        nc.vector.tensor_mul(mu2[:], mu[:], mu[:])
        nc.vector.tensor_tensor(out=var[:], in0=var[:], in1=mu2[:], op=ALU.subtract)
        rstd = ph1_st.tile([1, CH], F32, tag="rstd", name=f"rstd{c}")
        nc.scalar.activation(rstd[:], var[:], AF.Sqrt, bias=eps_t[:])
        nc.vector.reciprocal(rstd[:], rstd[:])
        nc.vector.tensor_mul(xs_tail[0:1, sl], mu[:], rstd[:])
        rstd_bf = ph1_st.tile([1, CH], BF16, tag="rstd_bf", name=f"rstd_bf{c}")
        nc.vector.tensor_copy(rstd_bf[:], rstd[:])
        rstd_b = ph1_st.tile([128, CH], BF16, tag="rstd_b", name=f"rstd_b{c}")
        nc.gpsimd.partition_broadcast(rstd_b[:], rstd_bf[:])
        for k in range(KT_C):
            nc.vector.tensor_mul(xs[k][:, sl], xbs[k], rstd_b[:])
        # qk for the previous chunk fills the PE while this chunk's stats drain
        if c > 0:
            qk_chunk(c - 1)
        # v for this chunk's token blocks: v_sb[tb] = [tok, head, d | ones]
        for tb in range(c * CH // 128, (c + 1) * CH // 128):
            tsl = slice(tb * 128, (tb + 1) * 128)
            for half in range(2):
                fsl = slice(half * (HL // 2) * (D + 1),
                            (half + 1) * (HL // 2) * (D + 1))
                vp = v_ps.tile([128, (HL // 2) * (D + 1)], F32, tag="v_ps",
                               name=f"vps{tb}_{half}")
                for k in range(KS):
                    lhs = xs[k][:, tsl] if k < KT_C else xs_tail[:, tsl]
                    nc.tensor.matmul(vp[:], _r(lhs), _r(wv_t[:, k, fsl]),
                                     start=(k == 0), stop=(k == KS - 1))
                nc.vector.tensor_copy(
                    v_sb[tb][:, half * (HL // 2):(half + 1) * (HL // 2), :], vp[:])
    qk_chunk(NCH - 1)

    # residual staging: x2[k] = bf16(xres[k] + b_proj[k]) (consumed in phase 3)
    for k in range(KT_C):
        xr = xr_in.tile([128, TO], F32, tag="xr", name=f"xr{k}")
        nc.sync.dma_start(xr[:], xres[k * 128:(k + 1) * 128, :])
        nc.vector.tensor_scalar_add(x2[k][:], xr[:], aux_sb[:, k:k + 1])

    for p in (xr_in, qk_ps, v_ps, ph1_st, ph1_ps, ph1_sq, ph1_in,
              wqkp, wvp, bigA):
        p.release()

    # ---------------- phase 2: attention + proj + split ReduceScatter ----------
    fc_w = tc.alloc_tile_pool(name="fc_w", bufs=14)    # fc1 weights; outlives
    bigY = tc.alloc_tile_pool(name="bigY", bufs=1)     # yT (lives thru proj)
    wpp = tc.alloc_tile_pool(name="wpp", bufs=1)
    sc_ps = tc.alloc_tile_pool(name="sc_ps", bufs=2, space="PSUM")
    ex_sb = tc.alloc_tile_pool(name="ex_sb", bufs=3)
    av_ps = tc.alloc_tile_pool(name="av_ps", bufs=1, space="PSUM")
    bc_ps = tc.alloc_tile_pool(name="bc_ps", bufs=1, space="PSUM")
    yn_sb = tc.alloc_tile_pool(name="yn_sb", bufs=2)
    pr_ps = tc.alloc_tile_pool(name="pr_ps", bufs=1, space="PSUM")
    pr_sb = tc.alloc_tile_pool(name="pr_sb", bufs=4)
    dram = tc.alloc_tile_pool(name="dram", bufs=1, space="DRAM")

    yT = [bigY.tile([128, T], BF16, tag=f"yT{k}", name=f"yT{k}")
          for k in range(HL * D // 128)]

    wp_t = wpp.tile([128, KT_P, C], BF16, tag="wp_main")
    nc.sync.dma_start(wp_t[:], wp.rearrange("(kt p) f -> p kt f", p=128))

    # rs_in[m] slot0 = chunk m partial, slot1 = chunk m+2 partial; the
    # pairwise ReduceScatter hands chunk m to the even core, m+2 to the odd.
    rs_in = [dram.tile([2, C, CH], BF16, tag=f"rsin{m}", name=f"rsin{m}")
             for m in range(2)]
    rs_out = [dram.tile([C, CH], BF16, tag=f"rsout{m}", name=f"rsout{m}")
              for m in range(2)]

    def fc1_wt(m, f):
        wt = fc_w.tile([128, KT_C, 128], BF16, tag="wfc_t", name=f"wfc{m}_{f}")
        nc.sync.dma_start(
            wt[:],
            wfc[:, f * 128:(f + 1) * 128].rearrange("(kt p) n -> p kt n", p=128))
        return wt

    fc1_pre = []

    # Schraudolph fast-exp constants for bf16: bitcast(int16(A*x + B)),
    # A folds the 1/sqrt(D) score scale; max rel err ~3.3%, which the softmax
    # row-normalization largely cancels (<=0.006 abs on attention out).
    SCHR_A = 184.664965 / np.sqrt(D)
    SCHR_B = 16250.4

    # chunk order (1,3,2,0): rs pair 1 (chunks 1,3) is issued at 60% of the
    # attention work and its transfer lands inside long chunk 2, where the
    # deferred normalize absorbs the collective's DMA freeze; rs pair 0
    # (chunks 0,2) goes at the end, hidden under phase 3's first half.
    for qc in (1, 3, 0, 2):
        qsl = slice(qc * CH, (qc + 1) * CH)
        n_kb = (qc + 1) * (CH // KB)      # diag blocks are the last 4
        if qc == 2:
            # prefetch fc1 weights for token-half 1 during the last chunk's
            # attention: regular DMA freezes while the final collective runs,
            # and these 14 tiles (~27us of matmul) keep the PE fed through it
            for f in range(14):
                fc1_pre.append(fc1_wt(1, f))
        norm = []                         # per-hp deferred normalize state
        for hp in range(HL // 2):
            yps = [av_ps.tile([128, CH], F32, tag=f"av{hh}",
                              name=f"av{hp}_{hh}_{qc}") for hh in range(2)]
            for kb in range(n_kb):
                diag_j = kb - (n_kb - CH // KB)
                q0 = max(0, diag_j * KB)
                ksl = slice(kb * KB, (kb + 1) * KB)
                # both heads' scores into one 2-bank PSUM tile, one batched exp
                sp = sc_ps.tile([128, 2, CH], F32, tag="sc",
                                name=f"sc{hp}_{qc}_{kb}")
                for hh in range(2):
                    prow = slice(hh * D, (hh + 1) * D)
                    nc.tensor.matmul(sp[:, hh, q0:], kT[hp][prow, ksl],
                                     qT[hp][prow, qsl][:, q0:],
                                     start=True, stop=True)
                if diag_j < 0 and kb % 3 == 2:
                    # off-diagonal block: Schraudolph exp on the DVE to
                    # unload the ACT engine (the attention pacer)
                    eti = ex_sb.tile([128, 2, CH], mybir.dt.int16, tag="exi",
                                     name=f"exi{hp}_{qc}_{kb}")
                    with nc.allow_low_precision(reason="schraudolph exp"):
                        nc.vector.tensor_scalar(
                            out=eti[:], in0=sp[:], scalar1=SCHR_A,
                            scalar2=SCHR_B, op0=ALU.mult, op1=ALU.add)
                    et_h = [eti[:, hh, :].bitcast(BF16) for hh in range(2)]
                else:
                    et = ex_sb.tile([128, 2, CH], BF16, tag="ex",
                                    name=f"ex{hp}_{qc}_{kb}")
                    nc.scalar.activation(et[:, :, q0:], sp[:, :, q0:],
                                         AF.Exp, scale=1.0 / np.sqrt(D))
                    if diag_j >= 0:
                        # causal mask on DVE (gpsimd is busy with the chunk
                        # ReduceScatter; affine_select there stalls AVs)
                        nc.vector.tensor_mul(
                            et[:, :, q0:q0 + KB], et[:, :, q0:q0 + KB],
                            masks[q0 // KB][:])
                    et_h = [et[:, hh, q0:] for hh in range(2)]
                for hh in range(2):
                    nc.tensor.matmul(yps[hh][:D + 1, q0:],
                                     v_sb[kb][:, hp * 2 + hh, :],
                                     et_h[hh],
                                     start=(kb == 0), stop=(kb == n_kb - 1))
            # stage y and issue the 1/rowsum chain now; the PE-side broadcast
            # and final muls run after the hp loop so the in-order PE queue
            # never parks behind this DVE/DMA latency chain
            ycop = [yn_sb.tile([D + 1, CH], BF16, tag=f"ycop{hh}_{hp}",
                               name=f"yc{hp}_{hh}_{qc}") for hh in range(2)]
            for hh in range(2):
                nc.vector.tensor_copy(ycop[hh][:], yps[hh][:D + 1, :])
            ypair_hi = yn_sb.tile([128, CH], BF16, tag=f"ypair_{hp}",
                                  name=f"yp{hp}_{qc}")
            nc.sync.dma_start(ypair_hi[D:2 * D, :], ycop[1][:D, :])
            # 1/rowsum: scatter each [1,512] sum row over 128 partitions so
            # the iterative-divide reciprocal runs at FD=8, then gather back.
            srp = yn_sb.tile([128, 2, CH // 128], BF16, tag=f"srp_{hp}",
                             name=f"srp{hp}_{qc}")
            srec = yn_sb.tile([2, CH], BF16, tag=f"srec_{hp}",
                              name=f"sr{hp}_{qc}")
            for hh in range(2):
                nc.sync.dma_start(srp[:, hh, :], ycop[hh][D:D + 1, :])
            with nc.allow_low_precision(reason="1/rowsum rounded to bf16"):
                nc.vector.reciprocal(srp[:], srp[:])
            for hh in range(2):
                nc.sync.dma_start(srec[hh:hh + 1, :], srp[:, hh, :])
            norm.append((ycop, ypair_hi, srec))
        for hp, (ycop, ypair_hi, srec) in enumerate(norm):
            bc = bc_ps.tile([128, CH], F32, tag="bc", name=f"bc{hp}_{qc}")
            nc.tensor.matmul(bc[:], sel2b[:], srec[:], start=True, stop=True)
            nc.vector.tensor_mul(yT[hp][0:D, qsl], ycop[0][:D, :], bc[0:D, :])
            nc.vector.tensor_mul(yT[hp][D:2 * D, qsl], ypair_hi[D:2 * D, :],
                                 bc[D:2 * D, :])
        # proj partial for this chunk; collective inputs go via the gpsimd
        # DMA queue so the Sync queue stays short for the softmax DMAs
        m, slot = qc % 2, qc // 2
        for ob in range(C // 128):
            osl = slice(ob * 128, (ob + 1) * 128)
            pp = pr_ps.tile([128, CH], F32, tag="pr", name=f"pr{qc}_{ob}")
            for k in range(KT_P):
                nc.tensor.matmul(pp[:], _r(wp_t[:, k, osl]), yT[k][:, qsl],
                                 start=(k == 0), stop=(k == KT_P - 1))
            pt = pr_sb.tile([128, CH], BF16, tag="pr_sb", name=f"prs{qc}_{ob}")
            nc.vector.tensor_copy(pt[:], pp[:])
            nc.gpsimd.dma_start(rs_in[m][slot, osl, :], pt[:])
        if qc >= 2:
            nc.gpsimd.collective_compute(
                "ReduceScatter", ALU.add,
                replica_groups=[[0, 1], [2, 3], [4, 5], [6, 7]],
                ins=[rs_in[m].opt()], outs=[rs_out[m].opt()])

    for p in (pr_sb, pr_ps, yn_sb, bc_ps, av_ps, ex_sb, sc_ps, wpp, bigY,
              bigQK, bigV):
        p.release()

    # ---------------- phase 3: residual + LN2 + MLP ----------------
    fc_ps = tc.alloc_tile_pool(name="fc_ps", bufs=3, space="PSUM")
    h_pool = tc.alloc_tile_pool(name="h_pool", bufs=1)
    out_sb = tc.alloc_tile_pool(name="out_sb", bufs=3)
    res_in = tc.alloc_tile_pool(name="res_in", bufs=2)
    ln2_ps = tc.alloc_tile_pool(name="ln2_ps", bufs=1, space="PSUM")
    ln2_sq = tc.alloc_tile_pool(name="ln2_sq", bufs=2)
    ln2_st = tc.alloc_tile_pool(name="ln2_st", bufs=1)

    xs2 = [bigX.tile([128, TO], BF16, tag=f"xs2_{k}", name=f"xs2_{k}")
           for k in range(KT_C)]
    hT_tail = h_pool.tile([128, CH], BF16, tag="h_tail")
    nc.vector.memset(hT_tail[:], 0)
    nc.vector.tensor_copy(hT_tail[0:1, :], ones_stage[0:1, :CH])
    hT = [[h_pool.tile([128, CH], BF16, tag=f"h{m}_{f}", name=f"hT{m}_{f}")
           for f in range(KT_F)] for m in range(2)]

    def ln2_stats(m):
        # residual add + LN2 stats; residual reads go on the Sync queue (the
        # gpsimd queue parks behind the pending ReduceScatter triggers)
        msl = slice(m * CH, (m + 1) * CH)
        for k in range(KT_C):
            rt = res_in.tile([128, CH], BF16, tag="rs_t", name=f"rst{m}_{k}")
            nc.sync.dma_start(rt[:], rs_out[m][k * 128:(k + 1) * 128, :])
            nc.vector.tensor_add(x2[k][:, msl], x2[k][:, msl], rt[:])
        sum_ps = ln2_ps.tile([1, CH], F32, tag="sum", name=f"l2sum{m}")
        sq_ps = ln2_ps.tile([1, CH], F32, tag="sq", name=f"l2sq{m}")
        for k in range(KT_C):
            x2q = ln2_sq.tile([128, CH], BF16, tag="x2", name=f"l2x2_{m}_{k}")
            nc.vector.tensor_mul(x2q[:], x2[k][:, msl], x2[k][:, msl])
            nc.tensor.matmul(sum_ps[:], ones_lhs_bf[:], x2[k][:, msl],
                             start=(k == 0), stop=(k == KT_C - 1))
            nc.tensor.matmul(sq_ps[:], ones_lhs_bf[:], x2q[:],
                             start=(k == 0), stop=(k == KT_C - 1))
        return sum_ps, sq_ps

    def ln2_norm(m, sum_ps, sq_ps):
        # broadcasts run as K=2 matmuls (the gpsimd partition_broadcast
        # would park behind the pending ReduceScatter trigger on that queue)
        msl = slice(m * CH, (m + 1) * CH)
        mu = ln2_st.tile([1, CH], F32, tag=f"mu{m}", name=f"l2mu{m}")
        nc.scalar.mul(mu[:], sum_ps[:], 1.0 / C)
        var = ln2_st.tile([1, CH], F32, tag=f"var{m}", name=f"l2var{m}")
        nc.scalar.mul(var[:], sq_ps[:], 1.0 / C)
        mu2 = ln2_st.tile([1, CH], F32, tag=f"mu2_{m}", name=f"l2mu2_{m}")
        nc.vector.tensor_mul(mu2[:], mu[:], mu[:])
        nc.vector.tensor_tensor(out=var[:], in0=var[:], in1=mu2[:], op=ALU.subtract)
        rstd = ln2_st.tile([1, CH], F32, tag=f"rstd{m}", name=f"l2rstd{m}")
        nc.scalar.activation(rstd[:], var[:], AF.Sqrt, bias=eps_t[:])
        nc.vector.reciprocal(rstd[:], rstd[:])
        nmr = ln2_st.tile([1, CH], F32, tag=f"nmr{m}", name=f"l2nmr{m}")
        nc.vector.tensor_mul(nmr[:], mu[:], rstd[:])
        rstd_b = ln2_ps.tile([128, CH], F32, tag="rb", name=f"l2rb{m}")
        nc.tensor.matmul(rstd_b[:], ones_row[:], rstd[:],
                         start=True, stop=True)
        nmr_b = ln2_ps.tile([128, CH], F32, tag="nb", name=f"l2nb{m}")
        nc.tensor.matmul(nmr_b[:], ones_row[:], nmr[:],
                         start=True, stop=True)
        for k in range(KT_C):
            tt = ln2_sq.tile([128, CH], BF16, tag="tt", name=f"l2t{m}_{k}")
            nc.vector.tensor_mul(tt[:], x2[k][:, msl], rstd_b[:])
            nc.vector.tensor_tensor(out=xs2[k][:, msl], in0=tt[:],
                                    in1=nmr_b[:], op=ALU.subtract)

    def fc1_chunk(m, pre=(), mid_hook=None):
        msl = slice(m * CH, (m + 1) * CH)
        for f in range(KT_F):
            if f == KT_F - 6 and mid_hook is not None:
                # other chunk's LN2 stats interleave into this chunk's tail
                # so its scalar/DVE chain hides under the last f-tiles
                mid_hook()
            wt = pre[f] if f < len(pre) else fc1_wt(m, f)
            hps = fc_ps.tile([128, CH], F32, tag="fc1", name=f"fc1p{m}_{f}")
            for k in range(KT_C):
                nc.tensor.matmul(hps[:], _r(wt[:, k, :]), xs2[k][:, msl],
                                 start=(k == 0), stop=(k == KT_C - 1))
            nc.scalar.activation(hT[m][f][:], hps[:], AF.Gelu_apprx_tanh,
                                 bias=aux_sb[:, 8 + f:9 + f])

    # token-half 1 first: its ReduceScatter was issued at 60% of attention,
    # so it is complete by now; half 0's collective hides under fc1_chunk(1)
    st1 = ln2_stats(1)
    ln2_norm(1, *st1)
    st0 = []

    def _stats0():
        st0.append(ln2_stats(0))

    fc1_chunk(1, pre=fc1_pre, mid_hook=_stats0)
    ln2_norm(0, *st0[0])
    fc1_chunk(0)
    for p in (ln2_st, ln2_sq, ln2_ps, res_in):
        p.release()

    fc2_w = tc.alloc_tile_pool(name="fc2_w", bufs=3)
    fc2_ps = tc.alloc_tile_pool(name="fc2_ps", bufs=2, space="PSUM")
    n_fg = (KT_F2 + FG - 1) // FG
    for ob in range(C // 128):
        osl = slice(ob * 128, (ob + 1) * 128)
        ops = [fc2_ps.tile([128, CH], F32, tag=f"fc2m{m}", name=f"fc2p{m}_{ob}")
               for m in range(2)]
        for fg in range(n_fg):
            lo, hi = fg * FG, min(fg * FG + FG, KT_F2)
            w2 = fc2_w.tile([128, FG, 128], BF16, tag="wfc2_t",
                            name=f"w2_{ob}_{fg}")
            nc.sync.dma_start(
                w2[:, : hi - lo, :],
                wfc2[lo * 128: hi * 128, osl]
                .rearrange("(kt p) n -> p kt n", p=128))
            for m in range(2):
                for j in range(lo, hi):
                    h_j = hT[m][j] if j < KT_F else hT_tail
                    nc.tensor.matmul(ops[m][:], _r(w2[:, j - lo, :]), _r(h_j[:]),
                                     start=(j == 0), stop=(j == KT_F2 - 1))
        for m in range(2):
            msl = slice(m * CH, (m + 1) * CH)
            ot = out_sb.tile([128, CH], F32, tag="ot", name=f"ot{m}_{ob}")
            nc.vector.tensor_add(ot[:], ops[m][:], x2[ob][:, msl])
            nc.sync.dma_start(out_ap[osl, msl], ot[:])

    for p in (fc2_ps, fc2_w, out_sb, h_pool, fc_ps, fc_w, dram, bigX, const):
        p.release()


_NC_CACHE = None


def _build():
    global _NC_CACHE
    if _NC_CACHE is not None:
        return _NC_CACHE
    nc = bacc.Bacc("TRN2", target_bir_lowering=False, debug=False, num_devices=8)
    io = {
        "xT": nc.dram_tensor("xT", [C, T], F32R, kind="ExternalInput").ap(),
        "xres": nc.dram_tensor("xres", [C, TO], F32, kind="ExternalInput").ap(),
        "wq": nc.dram_tensor("wq", [CP, HL * D], BF16, kind="ExternalInput").ap(),
        "wk": nc.dram_tensor("wk", [CP, HL * D], BF16, kind="ExternalInput").ap(),
        "wv": nc.dram_tensor("wv", [CP, HL * (D + 1)], BF16,
                             kind="ExternalInput").ap(),
        "wp": nc.dram_tensor("wp", [HL * D, C], BF16, kind="ExternalInput").ap(),
        "wfc": nc.dram_tensor("wfc", [C, FF], BF16, kind="ExternalInput").ap(),
        "wfc2": nc.dram_tensor("wfc2", [FP2, C], BF16, kind="ExternalInput").ap(),
        "sel": nc.dram_tensor("sel", [2, 128], F32R, kind="ExternalInput").ap(),
        "aux": nc.dram_tensor("aux", [128, 40], F32, kind="ExternalInput").ap(),
        "out": nc.dram_tensor("out", [C, TO], F32, kind="ExternalOutput").ap(),
    }
    with tile.TileContext(nc) as tc:
        _emit(tc, io)
    nc.compile()
    _NC_CACHE = nc
    return nc


def _stack_ln(w, g, b, bias):
    """[w*g ; -colsum(w*g) ; b@w + bias ; zero pad] -> [CP, F] float32."""
    wg = (w * g[:, None]).astype(np.float32)
    out = np.zeros((CP, w.shape[1]), np.float32)
    out[:C] = wg
    out[C] = -wg.sum(0)
    out[C + 1] = b @ w + bias
    return out


def kernel(x, ln1_g, ln1_b, w_attn, b_attn, w_proj, b_proj,
           ln2_g, ln2_b, w_fc, b_fc, w_fc2, b_fc2):
    f32 = lambda a: np.asarray(a, np.float32)
    x = f32(x)
    ln1_g, ln1_b, w_attn, b_attn = f32(ln1_g), f32(ln1_b), f32(w_attn), f32(b_attn)
    w_proj, b_proj, ln2_g, ln2_b = f32(w_proj), f32(b_proj), f32(ln2_g), f32(ln2_b)
    w_fc, b_fc, w_fc2, b_fc2 = f32(w_fc), f32(b_fc), f32(w_fc2), f32(b_fc2)

    nc = _build()

    qkv_stack = _stack_ln(w_attn, ln1_g, ln1_b, b_attn)        # [C+2, 3C]
    fc_w_g = (w_fc * ln2_g[:, None]).astype(np.float32)        # [C, FF]
    bfc_eff = ln2_b @ w_fc + b_fc                              # [FF]
    fc2_stack = np.zeros((FP2, C), np.float32)
    fc2_stack[:FF] = w_fc2
    fc2_stack[FF] = b_fc2

    sel_np = np.zeros((2, 128), np.float32)
    sel_np[0, :64] = 1.0
    sel_np[1, 64:] = 1.0

    aux_np = np.zeros((128, 40), np.float32)
    aux_np[:, :KT_C] = b_proj.reshape(KT_C, 128).T             # x2 staging bias
    aux_np[:, 8:8 + KT_F] = bfc_eff.reshape(KT_F, 128).T       # gelu bias

    in_maps = []
    for core in range(8):
        b_idx, r = core // 2, core % 2
        hsl = slice(r * HL * D, (r + 1) * HL * D)              # this core's heads
        xT_b = np.ascontiguousarray(x[b_idx].T)                # [C, T]
        wv_cols = qkv_stack[:, 2 * C + r * HL * D: 2 * C + (r + 1) * HL * D]
        wv_aug = np.zeros((CP, HL, D + 1), np.float32)
        wv_aug[:, :, :D] = wv_cols.reshape(CP, HL, D)
        wv_aug[C + 1, :, D] = 1.0                              # ones col via ones-row
        wp_loc = np.ascontiguousarray(w_proj[r * HL * D:(r + 1) * HL * D, :])
        bf = ml_dtypes.bfloat16
        in_maps.append({
            "xT": xT_b,
            "xres": np.ascontiguousarray(xT_b[:, r * TO:(r + 1) * TO]),
            "wq": np.ascontiguousarray(qkv_stack[:, hsl]).astype(bf),
            "wk": np.ascontiguousarray(
                qkv_stack[:, C + r * HL * D: C + (r + 1) * HL * D]).astype(bf),
            "wv": np.ascontiguousarray(
                wv_aug.reshape(CP, HL * (D + 1))).astype(bf),
            "wp": wp_loc.astype(bf),
            "wfc": fc_w_g.astype(bf),
            "wfc2": fc2_stack.astype(bf),
            "sel": sel_np,
            "aux": aux_np,
        })

    trace = bool(int(os.environ.get("KERNEL_TRACE", "0")))
    res = run_bass_kernel_spmd(nc, in_maps, core_ids=list(range(8)), trace=trace)
    kernel.last_result = res

    out = np.empty((B, T, C), np.float32)
    for core in range(8):
        b_idx, r = core // 2, core % 2
        out[b_idx, r * TO:(r + 1) * TO, :] = res.results[core]["out"].T
    return out


# revision 61
# speedup vs baseline: 1.1244x; 1.0600x over previous
"""Trainium2 Bass kernel for a dense transformer block (LN-attn-LN-MLP).

Sharding: core pair (2b, 2b+1) handles batch b. Each core computes 8 of the
16 attention heads over the full 2048-token sequence (head/tensor parallel),
then the pair ReduceScatters the partial c_proj output over tokens; the MLP
runs on each core's 1024-token half. All activations on chip are kept
feature-major [features, tokens] so no transposes are needed; the host
pre-transposes x and post-transposes the outputs.

Pipeline structure (v4):
 - Phase 1 prefetches the first x chunk ahead of the weights, computes LN1
   stats, q, k and v for all local heads, and stages the residual (bf16,
   with b_proj folded in) while the PE has slack.
 - Phase 2 runs attention query-chunk-major in order (1,3,0,2) so each pair
   ReduceScatter is issued as early as its two chunks allow; per chunk both
   heads of a pair share one PSUM score tile and a single batched exp
   ACTIVATE; ~1/3 of the off-diagonal exp blocks run on the DVE instead via
   a Schraudolph bit-trick exp (int16 bits -> bf16 bitcast) to unload the
   ACT engine that otherwise paces attention. The softmax 1/rowsum chains
   for all four head pairs are software-pipelined: reciprocal chains issue
   inside the head loop, the PE-side broadcast matmuls and the final muls
   run batched at chunk end so the in-order PE queue never waits on them.
   proj partials are written to the collective buffers on the gpsimd queue
   (keeps the Sync DMA queue short for the tiny softmax DMAs).
 - Phase 3 processes token-half 1 then 0 (matching collective completion
   order), adds the residual, runs LN2 (mean subtraction on DVE; gamma in
   the weights, beta/bias on the GELU activation bias) and the MLP per
   512-token chunk; fc1 contracts over exactly 8 k-tiles.
"""

import os
import sys

sys.path.insert(0, "/opt/trn_rl_repo")

import ml_dtypes
import numpy as np

import concourse.bass as bass
import concourse.tile as tile
from concourse import bacc, mybir
from concourse.bass_utils import run_bass_kernel_spmd

F32 = mybir.dt.float32
F32R = mybir.dt.float32r
BF16 = mybir.dt.bfloat16
AF = mybir.ActivationFunctionType
ALU = mybir.AluOpType

# Model dims
C = 1024            # embed
T = 2048            # sequence length (full context per core)
B = 4               # batch
NHEAD = 16
D = 64              # head dim
HL = 8              # local heads per core
TO = 1024           # output tokens per core (after pair ReduceScatter)
CH = 512            # token chunk (matmul free dim)
NCH = T // CH       # 4
KB = 128            # key block
FF = 4 * C          # 4096
EPS = 1e-5

KT_C = C // 128     # 8 k-tiles over embed dim
KS = KT_C + 1       # k-tiles incl. the mu'/ones/zero-pad tail tile (qkv only)
CP = KS * 128       # padded stack rows (1152)
KT_P = HL * D // 128       # proj k-tiles (4; bias folded into residual)
KT_F = FF // 128           # 32
KT_F2 = FF // 128 + 1      # fc2 k-tiles incl. tail (33)
FP2 = KT_F2 * 128          # padded fc2 rows (4224)
FG = 8                     # fc2 weight k-tiles per DMA group


def _r(ap):
    return ap.bitcast(F32R) if ap.dtype == F32 else ap


def _emit(tc, io):
    nc = tc.nc
    xT, xres, wq, wk, wv, wp, wfc, wfc2, out_ap = (
        io["xT"], io["xres"], io["wq"], io["wk"], io["wv"], io["wp"],
        io["wfc"], io["wfc2"], io["out"])

    const = tc.alloc_tile_pool(name="const", bufs=1)
    ones_stage = const.tile([2, T], F32)
    nc.vector.memset(ones_stage[:], 1.0)
    ones_lhs_s = const.tile([128, 1], F32)
    nc.vector.memset(ones_lhs_s[:], 1.0)
    ones_lhs = const.tile([128, 1], F32R)
    nc.vector.tensor_copy(ones_lhs[:], ones_lhs_s[:])
    ones_lhs_bf = const.tile([128, 1], BF16)
    nc.vector.tensor_copy(ones_lhs_bf[:], ones_lhs_s[:])
    sel2 = const.tile([2, 128], F32R)
    nc.sync.dma_start(sel2[:], io["sel"])
    sel2b = const.tile([2, 128], BF16, tag="sel2b")
    nc.vector.tensor_copy(sel2b[:], sel2[:].bitcast(F32))
    ones_row = const.tile([1, 128], F32, tag="ones_row")  # K=1 bcast lhsT
    nc.vector.memset(ones_row[:], 1.0)
    aux_sb = const.tile([128, 40], F32)
    nc.sync.dma_start(aux_sb[:], io["aux"])
    eps_t = const.tile([1, 1], F32)
    nc.vector.memset(eps_t[:], EPS)
    masks = []
    for i in range(CH // KB):
        mt = const.tile([128, 2, KB], BF16, tag=f"mask{i}")
        nc.vector.memset(mt[:], 1.0)
        nc.gpsimd.affine_select(mt[:], mt[:], pattern=[[0, 2], [1, KB]],
                                compare_op=ALU.is_ge, fill=0.0, base=0,
                                channel_multiplier=-1)
        masks.append(mt)

    # ---------------- phase 1: LN1 + q,k,v + residual staging ----------------
    bigX = tc.alloc_tile_pool(name="bigX", bufs=1, side="right")    # residual (thru end)
    bigV = tc.alloc_tile_pool(name="bigV", bufs=1, side="right")    # v (thru attn)
    bigQK = tc.alloc_tile_pool(name="bigQK", bufs=1, side="right")  # q,k (thru attn)
    bigA = tc.alloc_tile_pool(name="bigA", bufs=1)                  # xs (phase 1 only)
    wvp = tc.alloc_tile_pool(name="wvp", bufs=1)
    wqkp = tc.alloc_tile_pool(name="wqkp", bufs=1)
    ph1_sq = tc.alloc_tile_pool(name="ph1_sq", bufs=3)
    ph1_ps = tc.alloc_tile_pool(name="ph1_ps", bufs=1, space="PSUM")
    ph1_st = tc.alloc_tile_pool(name="ph1_st", bufs=1)
    v_ps = tc.alloc_tile_pool(name="v_ps", bufs=2, space="PSUM")
    qk_ps = tc.alloc_tile_pool(name="qk_ps", bufs=2, space="PSUM")
    xr_in = tc.alloc_tile_pool(name="xr_in", bufs=1)

    xs = [bigA.tile([128, T], BF16, tag=f"xs{k}", name=f"xs{k}") for k in range(KT_C)]
    xs_tail = bigA.tile([128, T], BF16, tag="xs_tail")   # row0 = mu', row1 = ones
    nc.vector.memset(xs_tail[:], 0)
    nc.vector.tensor_copy(xs_tail[0:2, :], ones_stage[:])
    v_sb = [bigV.tile([128, HL, D + 1], BF16, tag=f"v{tb}", name=f"v{tb}")
            for tb in range(T // 128)]
    qT = [bigQK.tile([128, T], BF16, tag=f"qT{hp}", name=f"qT{hp}")
          for hp in range(HL // 2)]
    kT = [bigQK.tile([128, T], BF16, tag=f"kT{hp}", name=f"kT{hp}")
          for hp in range(HL // 2)]
    x2 = [bigX.tile([128, TO], BF16, tag=f"x2_{k}", name=f"x2_{k}")
          for k in range(KT_C)]

    # prefetch chunk 0 of x first: the LN1 stats chain is the critical path
    # at kernel start, the weights below overlap it on the scalar DMA queue
    for k in range(KT_C):
        nc.sync.dma_start(xs[k][:, 0:CH], xT[k * 128:(k + 1) * 128, 0:CH])

    wv_t = wvp.tile([128, KS, HL * (D + 1)], BF16, tag="wv_main")
    nc.scalar.dma_start(wv_t[:], wv.rearrange("(kt p) f -> p kt f", p=128))
    wqk_t = []
    for hp in range(HL // 2):
        pair = []
        for kind, w_ap in enumerate((wq, wk)):
            wtile = wqkp.tile([128, KS, 128], BF16, tag=f"wqk{hp}_{kind}")
            nc.scalar.dma_start(
                wtile[:],
                w_ap[:, hp * 128:(hp + 1) * 128]
                .rearrange("(kt p) f -> p kt f", p=128))
            pair.append(wtile)
        wqk_t.append(pair)

    def qk_chunk(c):
        sl = slice(c * CH, (c + 1) * CH)
        for hp in range(HL // 2):
            for kind in range(2):
                qp = qk_ps.tile([128, CH], F32, tag="qk_ps",
                                name=f"qkps{hp}_{kind}_{c}")
                for k in range(KS):
                    rhs = xs[k][:, sl] if k < KT_C else xs_tail[:, sl]
                    nc.tensor.matmul(qp[:], _r(wqk_t[hp][kind][:, k, :]), _r(rhs),
                                     start=(k == 0), stop=(k == KS - 1))
                dest = qT[hp] if kind == 0 else kT[hp]
                nc.vector.tensor_copy(dest[:, sl], qp[:])

    for c in range(NCH):
        sl = slice(c * CH, (c + 1) * CH)
        xbs = []
        # x arrives bf16 and lands straight in the xs tiles (normalized in
        # place later): no staging cast, and the stats matmuls stream at
        # full bf16 rate directly off the DMA
        for k in range(KT_C):
            if c > 0:
                nc.sync.dma_start(xs[k][:, sl], xT[k * 128:(k + 1) * 128, sl])
            xbs.append(xs[k][:, sl])
        # 3) LN1 stats + normalize
        sum_ps = ph1_ps.tile([1, CH], F32, tag="sum", name=f"sum{c}")
        sq_ps = ph1_ps.tile([1, CH], F32, tag="sq", name=f"sq{c}")
        for k in range(KT_C):
            x2q = ph1_sq.tile([128, CH], BF16, tag="x2", name=f"x2_{c}_{k}")
            nc.vector.tensor_mul(x2q[:], xbs[k], xbs[k])
            nc.tensor.matmul(sum_ps[:], ones_lhs_bf[:], xbs[k],
                             start=(k == 0), stop=(k == KT_C - 1))
            nc.tensor.matmul(sq_ps[:], ones_lhs_bf[:], x2q[:],
                             start=(k == 0), stop=(k == KT_C - 1))
        mu = ph1_st.tile([1, CH], F32, tag="mu", name=f"mu{c}")
        nc.scalar.mul(mu[:], sum_ps[:], 1.0 / C)
        var = ph1_st.tile([1, CH], F32, tag="var", name=f"var{c}")
        nc.scalar.mul(var[:], sq_ps[:], 1.0 / C)
        mu2 = ph1_st.tile([1, CH], F32, tag="mu2", name=f"mu2_{c}")
        nc.vector.tensor_mul(mu2[:], mu[:], mu[:])
        nc.vector.tensor_tensor(out=var[:], in0=var[:], in1=mu2[:], op=ALU.subtract)
        rstd = ph1_st.tile([1, CH], F32, tag="rstd", name=f"rstd{c}")
        nc.scalar.activation(rstd[:], var[:], AF.Sqrt, bias=eps_t[:])
        nc.vector.reciprocal(rstd[:], rstd[:])
        nc.vector.tensor_mul(xs_tail[0:1, sl], mu[:], rstd[:])
        rstd_bf = ph1_st.tile([1, CH], BF16, tag="rstd_bf", name=f"rstd_bf{c}")
        nc.vector.tensor_copy(rstd_bf[:], rstd[:])
        rstd_b = ph1_st.tile([128, CH], BF16, tag="rstd_b", name=f"rstd_b{c}")
        nc.gpsimd.partition_broadcast(rstd_b[:], rstd_bf[:])
        for k in range(KT_C):
            nc.vector.tensor_mul(xs[k][:, sl], xbs[k], rstd_b[:])
        # qk for the previous chunk fills the PE while this chunk's stats drain
        if c > 0:
            qk_chunk(c - 1)
        # v for this chunk's token blocks: v_sb[tb] = [tok, head, d | ones]
        for tb in range(c * CH // 128, (c + 1) * CH // 128):
            tsl = slice(tb * 128, (tb + 1) * 128)
            for half in range(2):
                fsl = slice(half * (HL // 2) * (D + 1),
                            (half + 1) * (HL // 2) * (D + 1))
                vp = v_ps.tile([128, (HL // 2) * (D + 1)], F32, tag="v_ps",
                               name=f"vps{tb}_{half}")
                for k in range(KS):
                    lhs = xs[k][:, tsl] if k < KT_C else xs_tail[:, tsl]
                    nc.tensor.matmul(vp[:], _r(lhs), _r(wv_t[:, k, fsl]),
                                     start=(k == 0), stop=(k == KS - 1))
                nc.vector.tensor_copy(
                    v_sb[tb][:, half * (HL // 2):(half + 1) * (HL // 2), :], vp[:])
    qk_chunk(NCH - 1)

    # residual staging: x2[k] = bf16(xres[k] + b_proj[k]) (consumed in phase 3)
    for k in range(KT_C):
        xr = xr_in.tile([128, TO], BF16, tag="xr", name=f"xr{k}")
        nc.sync.dma_start(xr[:], xres[k * 128:(k + 1) * 128, :])
        nc.vector.tensor_scalar_add(x2[k][:], xr[:], aux_sb[:, k:k + 1])

    for p in (xr_in, qk_ps, v_ps, ph1_st, ph1_ps, ph1_sq,
              wqkp, wvp, bigA):
        p.release()

    # ---------------- phase 2: attention + proj + split ReduceScatter ----------
    fc_w = tc.alloc_tile_pool(name="fc_w", bufs=14)    # fc1 weights; outlives
    bigY = tc.alloc_tile_pool(name="bigY", bufs=1)     # yT (lives thru proj)
    wpp = tc.alloc_tile_pool(name="wpp", bufs=1)
    sc_ps = tc.alloc_tile_pool(name="sc_ps", bufs=2, space="PSUM")
    ex_sb = tc.alloc_tile_pool(name="ex_sb", bufs=3)
    av_ps = tc.alloc_tile_pool(name="av_ps", bufs=1, space="PSUM")
    bc_ps = tc.alloc_tile_pool(name="bc_ps", bufs=1, space="PSUM")
    yn_sb = tc.alloc_tile_pool(name="yn_sb", bufs=2)
    pr_ps = tc.alloc_tile_pool(name="pr_ps", bufs=1, space="PSUM")
    pr_sb = tc.alloc_tile_pool(name="pr_sb", bufs=4)
    dram = tc.alloc_tile_pool(name="dram", bufs=1, space="DRAM")

    yT = [bigY.tile([128, T], BF16, tag=f"yT{k}", name=f"yT{k}")
          for k in range(HL * D // 128)]

    wp_t = wpp.tile([128, KT_P, C], BF16, tag="wp_main")
    nc.sync.dma_start(wp_t[:], wp.rearrange("(kt p) f -> p kt f", p=128))

    # rs_in[m] slot0 = chunk m partial, slot1 = chunk m+2 partial; the
    # pairwise ReduceScatter hands chunk m to the even core, m+2 to the odd.
    rs_in = [dram.tile([2, C, CH], BF16, tag=f"rsin{m}", name=f"rsin{m}")
             for m in range(2)]
    rs_out = [dram.tile([C, CH], BF16, tag=f"rsout{m}", name=f"rsout{m}")
              for m in range(2)]

    def fc1_wt(m, f):
        wt = fc_w.tile([128, KT_C, 128], BF16, tag="wfc_t", name=f"wfc{m}_{f}")
        nc.sync.dma_start(
            wt[:],
            wfc[:, f * 128:(f + 1) * 128].rearrange("(kt p) n -> p kt n", p=128))
        return wt

    fc1_pre = []

    # Schraudolph fast-exp constants for bf16: bitcast(int16(A*x + B)),
    # A folds the 1/sqrt(D) score scale; max rel err ~3.3%, which the softmax
    # row-normalization largely cancels (<=0.006 abs on attention out).
    SCHR_A = 184.664965 / np.sqrt(D)
    SCHR_B = 16250.4

    # chunk order (1,3,2,0): rs pair 1 (chunks 1,3) is issued at 60% of the
    # attention work and its transfer lands inside long chunk 2, where the
    # deferred normalize absorbs the collective's DMA freeze; rs pair 0
    # (chunks 0,2) goes at the end, hidden under phase 3's first half.
    for qc in (1, 3, 0, 2):
        qsl = slice(qc * CH, (qc + 1) * CH)
        n_kb = (qc + 1) * (CH // KB)      # diag blocks are the last 4
        if qc == 2:
            # prefetch fc1 weights for token-half 1 during the last chunk's
            # attention: regular DMA freezes while the final collective runs,
            # and these 14 tiles (~27us of matmul) keep the PE fed through it
            for f in range(14):
                fc1_pre.append(fc1_wt(1, f))
        norm = []                         # per-hp deferred normalize state
        for hp in range(HL // 2):
            yps = [av_ps.tile([128, CH], F32, tag=f"av{hh}",
                              name=f"av{hp}_{hh}_{qc}") for hh in range(2)]
            for kb in range(n_kb):
                diag_j = kb - (n_kb - CH // KB)
                q0 = max(0, diag_j * KB)
                ksl = slice(kb * KB, (kb + 1) * KB)
                # both heads' scores into one 2-bank PSUM tile, one batched exp
                sp = sc_ps.tile([128, 2, CH], F32, tag="sc",
                                name=f"sc{hp}_{qc}_{kb}")
                for hh in range(2):
                    prow = slice(hh * D, (hh + 1) * D)
                    nc.tensor.matmul(sp[:, hh, q0:], kT[hp][prow, ksl],
                                     qT[hp][prow, qsl][:, q0:],
                                     start=True, stop=True)
                if diag_j < 0 and kb % 3 == 2:
                    # off-diagonal block: Schraudolph exp on the DVE to
                    # unload the ACT engine (the attention pacer)
                    eti = ex_sb.tile([128, 2, CH], mybir.dt.int16, tag="exi",
                                     name=f"exi{hp}_{qc}_{kb}")
                    with nc.allow_low_precision(reason="schraudolph exp"):
                        nc.vector.tensor_scalar(
                            out=eti[:], in0=sp[:], scalar1=SCHR_A,
                            scalar2=SCHR_B, op0=ALU.mult, op1=ALU.add)
                    et_h = [eti[:, hh, :].bitcast(BF16) for hh in range(2)]
                else:
                    et = ex_sb.tile([128, 2, CH], BF16, tag="ex",
                                    name=f"ex{hp}_{qc}_{kb}")
                    nc.scalar.activation(et[:, :, q0:], sp[:, :, q0:],
                                         AF.Exp, scale=1.0 / np.sqrt(D))
                    if diag_j >= 0:
                        # causal mask on DVE (gpsimd is busy with the chunk
                        # ReduceScatter; affine_select there stalls AVs)
                        nc.vector.tensor_mul(
                            et[:, :, q0:q0 + KB], et[:, :, q0:q0 + KB],
                            masks[q0 // KB][:])
                    et_h = [et[:, hh, q0:] for hh in range(2)]
                for hh in range(2):
                    nc.tensor.matmul(yps[hh][:D + 1, q0:],
                                     v_sb[kb][:, hp * 2 + hh, :],
                                     et_h[hh],
                                     start=(kb == 0), stop=(kb == n_kb - 1))
            # stage y and issue the 1/rowsum chain now; the PE-side broadcast
            # and final muls run after the hp loop so the in-order PE queue
            # never parks behind this DVE/DMA latency chain
            ycop = [yn_sb.tile([D + 1, CH], BF16, tag=f"ycop{hh}_{hp}",
                               name=f"yc{hp}_{hh}_{qc}") for hh in range(2)]
            for hh in range(2):
                nc.vector.tensor_copy(ycop[hh][:], yps[hh][:D + 1, :])
            ypair_hi = yn_sb.tile([128, CH], BF16, tag=f"ypair_{hp}",
                                  name=f"yp{hp}_{qc}")
            nc.sync.dma_start(ypair_hi[D:2 * D, :], ycop[1][:D, :])
            # 1/rowsum: scatter each [1,512] sum row over 128 partitions so
            # the iterative-divide reciprocal runs at FD=8, then gather back.
            srp = yn_sb.tile([128, 2, CH // 128], BF16, tag=f"srp_{hp}",
                             name=f"srp{hp}_{qc}")
            srec = yn_sb.tile([2, CH], BF16, tag=f"srec_{hp}",
                              name=f"sr{hp}_{qc}")
            for hh in range(2):
                nc.sync.dma_start(srp[:, hh, :], ycop[hh][D:D + 1, :])
            with nc.allow_low_precision(reason="1/rowsum rounded to bf16"):
                nc.vector.reciprocal(srp[:], srp[:])
            for hh in range(2):
                nc.sync.dma_start(srec[hh:hh + 1, :], srp[:, hh, :])
            norm.append((ycop, ypair_hi, srec))
        for hp, (ycop, ypair_hi, srec) in enumerate(norm):
            bc = bc_ps.tile([128, CH], F32, tag="bc", name=f"bc{hp}_{qc}")
            nc.tensor.matmul(bc[:], sel2b[:], srec[:], start=True, stop=True)
            nc.vector.tensor_mul(yT[hp][0:D, qsl], ycop[0][:D, :], bc[0:D, :])
            nc.vector.tensor_mul(yT[hp][D:2 * D, qsl], ypair_hi[D:2 * D, :],
                                 bc[D:2 * D, :])
        # proj partial for this chunk; collective inputs go via the gpsimd
        # DMA queue so the Sync queue stays short for the softmax DMAs
        m, slot = qc % 2, qc // 2
        for ob in range(C // 128):
            osl = slice(ob * 128, (ob + 1) * 128)
            pp = pr_ps.tile([128, CH], F32, tag="pr", name=f"pr{qc}_{ob}")
            for k in range(KT_P):
                nc.tensor.matmul(pp[:], _r(wp_t[:, k, osl]), yT[k][:, qsl],
                                 start=(k == 0), stop=(k == KT_P - 1))
            pt = pr_sb.tile([128, CH], BF16, tag="pr_sb", name=f"prs{qc}_{ob}")
            nc.vector.tensor_copy(pt[:], pp[:])
            nc.gpsimd.dma_start(rs_in[m][slot, osl, :], pt[:])
        if qc >= 2:
            nc.gpsimd.collective_compute(
                "ReduceScatter", ALU.add,
                replica_groups=[[0, 1], [2, 3], [4, 5], [6, 7]],
                ins=[rs_in[m].opt()], outs=[rs_out[m].opt()])

    for p in (pr_sb, pr_ps, yn_sb, bc_ps, av_ps, ex_sb, sc_ps, wpp, bigY,
              bigQK, bigV):
        p.release()

    # ---------------- phase 3: residual + LN2 + MLP ----------------
    fc_ps = tc.alloc_tile_pool(name="fc_ps", bufs=3, space="PSUM")
    h_pool = tc.alloc_tile_pool(name="h_pool", bufs=1)
    out_sb = tc.alloc_tile_pool(name="out_sb", bufs=3)
    res_in = tc.alloc_tile_pool(name="res_in", bufs=2)
    ln2_ps = tc.alloc_tile_pool(name="ln2_ps", bufs=1, space="PSUM")
    ln2_sq = tc.alloc_tile_pool(name="ln2_sq", bufs=2)
    ln2_st = tc.alloc_tile_pool(name="ln2_st", bufs=1)

    xs2 = [bigX.tile([128, TO], BF16, tag=f"xs2_{k}", name=f"xs2_{k}")
           for k in range(KT_C)]
    hT_tail = h_pool.tile([128, CH], BF16, tag="h_tail")
    nc.vector.memset(hT_tail[:], 0)
    nc.vector.tensor_copy(hT_tail[0:1, :], ones_stage[0:1, :CH])
    hT = [[h_pool.tile([128, CH], BF16, tag=f"h{m}_{f}", name=f"hT{m}_{f}")
           for f in range(KT_F)] for m in range(2)]

    def ln2_stats(m):
        # residual add + LN2 stats; residual reads go on the Sync queue (the
        # gpsimd queue parks behind the pending ReduceScatter triggers)
        msl = slice(m * CH, (m + 1) * CH)
        for k in range(KT_C):
            rt = res_in.tile([128, CH], BF16, tag="rs_t", name=f"rst{m}_{k}")
            nc.sync.dma_start(rt[:], rs_out[m][k * 128:(k + 1) * 128, :])
            nc.vector.tensor_add(x2[k][:, msl], x2[k][:, msl], rt[:])
        sum_ps = ln2_ps.tile([1, CH], F32, tag="sum", name=f"l2sum{m}")
        sq_ps = ln2_ps.tile([1, CH], F32, tag="sq", name=f"l2sq{m}")
        for k in range(KT_C):
            x2q = ln2_sq.tile([128, CH], BF16, tag="x2", name=f"l2x2_{m}_{k}")
            nc.vector.tensor_mul(x2q[:], x2[k][:, msl], x2[k][:, msl])
            nc.tensor.matmul(sum_ps[:], ones_lhs_bf[:], x2[k][:, msl],
                             start=(k == 0), stop=(k == KT_C - 1))
            nc.tensor.matmul(sq_ps[:], ones_lhs_bf[:], x2q[:],
                             start=(k == 0), stop=(k == KT_C - 1))
        return sum_ps, sq_ps

    def ln2_norm(m, sum_ps, sq_ps):
        # broadcasts run as K=2 matmuls (the gpsimd partition_broadcast
        # would park behind the pending ReduceScatter trigger on that queue)
        msl = slice(m * CH, (m + 1) * CH)
        mu = ln2_st.tile([1, CH], F32, tag=f"mu{m}", name=f"l2mu{m}")
        nc.scalar.mul(mu[:], sum_ps[:], 1.0 / C)
        var = ln2_st.tile([1, CH], F32, tag=f"var{m}", name=f"l2var{m}")
        nc.scalar.mul(var[:], sq_ps[:], 1.0 / C)
        mu2 = ln2_st.tile([1, CH], F32, tag=f"mu2_{m}", name=f"l2mu2_{m}")
        nc.vector.tensor_mul(mu2[:], mu[:], mu[:])
        nc.vector.tensor_tensor(out=var[:], in0=var[:], in1=mu2[:], op=ALU.subtract)
        rstd = ln2_st.tile([1, CH], F32, tag=f"rstd{m}", name=f"l2rstd{m}")
        nc.scalar.activation(rstd[:], var[:], AF.Sqrt, bias=eps_t[:])
        nc.vector.reciprocal(rstd[:], rstd[:])
        nmr = ln2_st.tile([1, CH], F32, tag=f"nmr{m}", name=f"l2nmr{m}")
        nc.vector.tensor_mul(nmr[:], mu[:], rstd[:])
        rstd_b = ln2_ps.tile([128, CH], F32, tag="rb", name=f"l2rb{m}")
        nc.tensor.matmul(rstd_b[:], ones_row[:], rstd[:],
                         start=True, stop=True)
        nmr_b = ln2_ps.tile([128, CH], F32, tag="nb", name=f"l2nb{m}")
        nc.tensor.matmul(nmr_b[:], ones_row[:], nmr[:],
                         start=True, stop=True)
        for k in range(KT_C):
            tt = ln2_sq.tile([128, CH], BF16, tag="tt", name=f"l2t{m}_{k}")
            nc.vector.tensor_mul(tt[:], x2[k][:, msl], rstd_b[:])
            nc.vector.tensor_tensor(out=xs2[k][:, msl], in0=tt[:],
                                    in1=nmr_b[:], op=ALU.subtract)

    def fc1_chunk(m, pre=(), mid_hook=None):
        msl = slice(m * CH, (m + 1) * CH)
        for f in range(KT_F):
            if f == KT_F - 6 and mid_hook is not None:
                # other chunk's LN2 stats interleave into this chunk's tail
                # so its scalar/DVE chain hides under the last f-tiles
                mid_hook()
            wt = pre[f] if f < len(pre) else fc1_wt(m, f)
            hps = fc_ps.tile([128, CH], F32, tag="fc1", name=f"fc1p{m}_{f}")
            for k in range(KT_C):
                nc.tensor.matmul(hps[:], _r(wt[:, k, :]), xs2[k][:, msl],
                                 start=(k == 0), stop=(k == KT_C - 1))
            nc.scalar.activation(hT[m][f][:], hps[:], AF.Gelu_apprx_tanh,
                                 bias=aux_sb[:, 8 + f:9 + f])

    # token-half 1 first: its ReduceScatter was issued at 60% of attention,
    # so it is complete by now; half 0's collective hides under fc1_chunk(1)
    st1 = ln2_stats(1)
    ln2_norm(1, *st1)
    st0 = []

    def _stats0():
        st0.append(ln2_stats(0))

    fc1_chunk(1, pre=fc1_pre, mid_hook=_stats0)
    ln2_norm(0, *st0[0])
    fc1_chunk(0)
    for p in (ln2_st, ln2_sq, ln2_ps, res_in):
        p.release()

    fc2_w = tc.alloc_tile_pool(name="fc2_w", bufs=3)
    fc2_ps = tc.alloc_tile_pool(name="fc2_ps", bufs=2, space="PSUM")
    n_fg = (KT_F2 + FG - 1) // FG
    for ob in range(C // 128):
        osl = slice(ob * 128, (ob + 1) * 128)
        ops = [fc2_ps.tile([128, CH], F32, tag=f"fc2m{m}", name=f"fc2p{m}_{ob}")
               for m in range(2)]
        for fg in range(n_fg):
            lo, hi = fg * FG, min(fg * FG + FG, KT_F2)
            w2 = fc2_w.tile([128, FG, 128], BF16, tag="wfc2_t",
                            name=f"w2_{ob}_{fg}")
            nc.sync.dma_start(
                w2[:, : hi - lo, :],
                wfc2[lo * 128: hi * 128, osl]
                .rearrange("(kt p) n -> p kt n", p=128))
            for m in range(2):
                for j in range(lo, hi):
                    h_j = hT[m][j] if j < KT_F else hT_tail
                    nc.tensor.matmul(ops[m][:], _r(w2[:, j - lo, :]), _r(h_j[:]),
                                     start=(j == 0), stop=(j == KT_F2 - 1))
        for m in range(2):
            msl = slice(m * CH, (m + 1) * CH)
            ot = out_sb.tile([128, CH], F32, tag="ot", name=f"ot{m}_{ob}")
            nc.vector.tensor_add(ot[:], ops[m][:], x2[ob][:, msl])
            nc.sync.dma_start(out_ap[osl, msl], ot[:])

    for p in (fc2_ps, fc2_w, out_sb, h_pool, fc_ps, fc_w, dram, bigX, const):
        p.release()


_NC_CACHE = None


def _build():
    global _NC_CACHE
    if _NC_CACHE is not None:
        return _NC_CACHE
    nc = bacc.Bacc("TRN2", target_bir_lowering=False, debug=False, num_devices=8)
    io = {
        "xT": nc.dram_tensor("xT", [C, T], BF16, kind="ExternalInput").ap(),
        "xres": nc.dram_tensor("xres", [C, TO], BF16, kind="ExternalInput").ap(),
        "wq": nc.dram_tensor("wq", [CP, HL * D], BF16, kind="ExternalInput").ap(),
        "wk": nc.dram_tensor("wk", [CP, HL * D], BF16, kind="ExternalInput").ap(),
        "wv": nc.dram_tensor("wv", [CP, HL * (D + 1)], BF16,
                             kind="ExternalInput").ap(),
        "wp": nc.dram_tensor("wp", [HL * D, C], BF16, kind="ExternalInput").ap(),
        "wfc": nc.dram_tensor("wfc", [C, FF], BF16, kind="ExternalInput").ap(),
        "wfc2": nc.dram_tensor("wfc2", [FP2, C], BF16, kind="ExternalInput").ap(),
        "sel": nc.dram_tensor("sel", [2, 128], F32R, kind="ExternalInput").ap(),
        "aux": nc.dram_tensor("aux", [128, 40], F32, kind="ExternalInput").ap(),
        "out": nc.dram_tensor("out", [C, TO], F32, kind="ExternalOutput").ap(),
    }
    with tile.TileContext(nc) as tc:
        _emit(tc, io)
    nc.compile()
    _NC_CACHE = nc
    return nc


def _stack_ln(w, g, b, bias):
    """[w*g ; -colsum(w*g) ; b@w + bias ; zero pad] -> [CP, F] float32."""
    wg = (w * g[:, None]).astype(np.float32)
    out = np.zeros((CP, w.shape[1]), np.float32)
    out[:C] = wg
    out[C] = -wg.sum(0)
    out[C + 1] = b @ w + bias
    return out


def kernel(x, ln1_g, ln1_b, w_attn, b_attn, w_proj, b_proj,
           ln2_g, ln2_b, w_fc, b_fc, w_fc2, b_fc2):
    f32 = lambda a: np.asarray(a, np.float32)
    x = f32(x)
    ln1_g, ln1_b, w_attn, b_attn = f32(ln1_g), f32(ln1_b), f32(w_attn), f32(b_attn)
    w_proj, b_proj, ln2_g, ln2_b = f32(w_proj), f32(b_proj), f32(ln2_g), f32(ln2_b)
    w_fc, b_fc, w_fc2, b_fc2 = f32(w_fc), f32(b_fc), f32(w_fc2), f32(b_fc2)

    nc = _build()

    qkv_stack = _stack_ln(w_attn, ln1_g, ln1_b, b_attn)        # [C+2, 3C]
    fc_w_g = (w_fc * ln2_g[:, None]).astype(np.float32)        # [C, FF]
    bfc_eff = ln2_b @ w_fc + b_fc                              # [FF]
    fc2_stack = np.zeros((FP2, C), np.float32)
    fc2_stack[:FF] = w_fc2
    fc2_stack[FF] = b_fc2

    sel_np = np.zeros((2, 128), np.float32)
    sel_np[0, :64] = 1.0
    sel_np[1, 64:] = 1.0

    aux_np = np.zeros((128, 40), np.float32)
    aux_np[:, :KT_C] = b_proj.reshape(KT_C, 128).T             # x2 staging bias
    aux_np[:, 8:8 + KT_F] = bfc_eff.reshape(KT_F, 128).T       # gelu bias

    in_maps = []
    bf = ml_dtypes.bfloat16
    for core in range(8):
        b_idx, r = core // 2, core % 2
        hsl = slice(r * HL * D, (r + 1) * HL * D)              # this core's heads
        xT_b = np.ascontiguousarray(x[b_idx].T).astype(bf)     # [C, T] bf16
        wv_cols = qkv_stack[:, 2 * C + r * HL * D: 2 * C + (r + 1) * HL * D]
        wv_aug = np.zeros((CP, HL, D + 1), np.float32)
        wv_aug[:, :, :D] = wv_cols.reshape(CP, HL, D)
        wv_aug[C + 1, :, D] = 1.0                              # ones col via ones-row
        wp_loc = np.ascontiguousarray(w_proj[r * HL * D:(r + 1) * HL * D, :])
        in_maps.append({
            "xT": xT_b,
            "xres": np.ascontiguousarray(xT_b[:, r * TO:(r + 1) * TO]),
            "wq": np.ascontiguousarray(qkv_stack[:, hsl]).astype(bf),
            "wk": np.ascontiguousarray(
                qkv_stack[:, C + r * HL * D: C + (r + 1) * HL * D]).astype(bf),
            "wv": np.ascontiguousarray(
                wv_aug.reshape(CP, HL * (D + 1))).astype(bf),
            "wp": wp_loc.astype(bf),
            "wfc": fc_w_g.astype(bf),
            "wfc2": fc2_stack.astype(bf),
            "sel": sel_np,
            "aux": aux_np,
        })

    trace = bool(int(os.environ.get("KERNEL_TRACE", "0")))
    res = run_bass_kernel_spmd(nc, in_maps, core_ids=list(range(8)), trace=trace)
    kernel.last_result = res

    out = np.empty((B, T, C), np.float32)
    for core in range(8):
        b_idx, r = core // 2, core % 2
        out[b_idx, r * TO:(r + 1) * TO, :] = res.results[core]["out"].T
    return out
